# revision 18
# baseline (speedup 1.0000x reference)
"""DGCNN forward kernel for Trainium2 (8 NeuronCores, data-parallel over batch).

Contract: kernel(**inputs) takes the FULL unsharded inputs (keyed as in
setup_inputs()) and returns the FULL (8, 3) float32 output.

Strategy
--------
B = 8 samples -> 1 sample per NeuronCore (pure data parallel; the tiny weights
are replicated). Per sample, the dominant work is stage 1 of the DGCNN:

  y     = x.reshape(3, 4096)            (flat view, matches the torch .view)
  dist  = (2*y^T y - xx_n) - xx_m       (4096 x 4096)
  idx   = top-3 columns per row         (includes self)
  x1    = leaky(bn1(max_n w1 @ [x[idx]-x[n]; x[n]]))   -> (64, 3)

Everything through the segment-max runs on device; bn1+leaky (monotone, so it
commutes with the max bit-exactly) and the later KNN stages on 64/6-point
clouds + the tiny MLP (~2 KFLOP total, <0.01% of the FLOPs) run on host in
float32, mirroring the reference ops exactly.

Distance matmul: operands are split exactly into hi+lo FP32R pieces (<=12
significant bits each, so every product is exact) and folded into one K=16
FP32R matmul per 512-column chunk, accumulated in PSUM in the reference's
rounding order.

Selection uses a QUAD-REDUCTION: columns {j, j+1024, j+2048, j+3072} form a
group. The PE emits S = distA+distB (cols 0:2048) and D = distA-distB (cols
2048:4096) for the (j, j+2048) pairs; ACT evicts 0.5*S (Copy) and 0.5*|D|
(Abs); one gpsimd CCE DMA (accum add) forms the pair key
m[j] = max(distA, distB); one DVE tensor_max folds pairs (j, j+1024) into
quad keys q[j] = max of the 4 group members. The DVE top-8/index scans then
run on 1024 keys instead of 4096 (the scans are the DVE bottleneck; they are
elem-count-bound at ~1/cycle regardless of dtype).

Exactness: the top-3 quads by q provably contain {self, nn1, nn2} (a quad
outranks nn2's quad only if it holds self or nn1). All 4 members of the top-3
quads are gathered as 96B rows [Ycol x4 | xrow x4] (SWDGE); the refine
recomputes exact squared distances from the FLAT-VIEW COLUMN coords Y (the
reference's distance space) while features use the flat-view ROW coords x
(the reference's gather space). The self candidate has d = -0.0 exactly, so
it is always refine-rank 0; ranks 1,2 are the k=1,2 neighbors, selected by a
one-hot sum over 12 slots. Numpy-validated: 0/32768 neighbor mismatches vs
the fp32 reference on the test data.

The reference conv contracts w1 against a FLAT .view() of the n-major
edge-feature stream (g.reshape(6, 12288) crosses point boundaries), so the
per-tile g rows ([d_kk(3) | x_n(3)] x 3, with d exactly fp32-subtracted)
round-trip through a DRAM scratch buffer whose flat re-view feeds plain
fp32 K=6 matmuls (exact; the PE has slack). Chunk maxima respect the k'
quarters and fold into the (64, 3) segment maxima at the end.

Per 128-row tile, software-pipelined (stage2 lags stage1 by LAG=3 tiles):
  PE    : 8 K=16 FP32R dist matmuls -> PSUM (S|D), fp32 conv matmuls
  ACT   : PSUM evict (Copy 0.5*S, Abs 0.5*D), refine square, g x-blocks
  DMA   : gpsimd CCE add m += tt; SWDGE candidate gathers; g stream store+load
  DVE   : quad tensor_max + InstMax/InstMaxIndex on q (1024) + refine scans
          + conv reduce_max
  GPSIMD: gathers + refine smalls (sub, one-hot select)
"""

import numpy as np

N = 4096
P = 128
NT = N // P           # 32 row tiles
B = 8
EPS = 1e-5
K = 3

_compiled = None


def _build(reps=1, ablate=None):
    # ablate: None=full, "dist"=PE+ACT only, "m"=+gpsimd pair-max,
    #         "q"=+DVE quad-max, "max"=+InstMax, "idx"=+InstMaxIndex,
    #         "cand"=+gathers, "dm"=+refine dist, "dsf"=+refine scans,
    #         "xs"=+one-hot select (no conv/transpose)
    import contextlib
    HCDBG = globals().get('HCDBG_COL', 128)

    import concourse.bass as bass
    import concourse.mybir as mybir
    from concourse import bacc
    from concourse.tile import TileContext

    f32 = mybir.dt.float32
    f32r = mybir.dt.float32r
    u32 = mybir.dt.uint32
    Copy = mybir.ActivationFunctionType.Copy
    H = N // 2
    Q = N // 4

    nc = bacc.Bacc(
        "TRN2", target_bir_lowering=False, debug=False, num_devices=B
    )
    lt = nc.declare_dram_parameter("lt", [16, N], f32r, isOutput=False)
    rt = nc.declare_dram_parameter("rt", [16, N], f32r, isOutput=False)
    xr = nc.declare_dram_parameter("xrows", [N, 3], f32, isOutput=False)
    xq = nc.declare_dram_parameter("xquad", [Q, 24], f32, isOutput=False)
    yc = nc.declare_dram_parameter("ycols", [N, 3], f32, isOutput=False)
    io16 = nc.declare_dram_parameter("iota16", [P, 16], f32, isOutput=False)
    w6p = nc.declare_dram_parameter("w6", [6, 64], f32r, isOutput=False)
    out_p = nc.declare_dram_parameter("out", [64, 3], f32, isOutput=True)

    g_sep = nc.dram_tensor("g_scratch", [N, 18], f32r)

    with TileContext(nc) as tc:
        with (
            tc.tile_pool(name="const", bufs=1) as cpool,
            tc.tile_pool(name="dist", bufs=4) as dpool,
            tc.tile_pool(name="mkey", bufs=6) as mpool,
            tc.tile_pool(name="work", bufs=10) as wpool,
        ):
            lt_sb = cpool.tile([16, N], f32r)
            nc.sync.dma_start(out=lt_sb[:, 0:256], in_=lt[:, 0:256])
            nc.sync.dma_start(out=lt_sb[:, 256:N], in_=lt[:, 256:N])
            rt_sb = cpool.tile([16, N], f32r)
            for cchunk in range(4):
                nc.sync.dma_start(
                    out=rt_sb[:, cchunk * 1024:(cchunk + 1) * 1024],
                    in_=rt[:, cchunk * 1024:(cchunk + 1) * 1024],
                )
            w6_sb = cpool.tile([6, 64], f32r)
            nc.sync.dma_start(out=w6_sb[:], in_=w6p[:])
            xall = cpool.tile([P, NT, 3], f32)
            nc.sync.dma_start(
                out=xall[:],
                in_=xr[:].rearrange("(t p) c -> p t c", p=P),
            )
            yall = cpool.tile([P, NT, 3], f32)
            nc.sync.dma_start(
                out=yall[:],
                in_=yc[:].rearrange("(t p) c -> p t c", p=P),
            )
            iota16 = cpool.tile([P, 16], f32)
            nc.sync.dma_start(out=iota16[:], in_=io16[:])

            loop_cm = tc.For_i(0, reps, 1) if reps > 1 else contextlib.nullcontext()
            with loop_cm:
              with tc.tile_pool(name="psum_d", bufs=3, space="PSUM") as ppool, \
                   tc.tile_pool(name="psum_c", bufs=1, space="PSUM") as cppool:
                partial2a = wpool.tile([64, 24], f32, tag="partial2a")
                # pre-initialize ring-buffer slots the steady-state loop
                # never writes: dm[:, 12:16] = -inf pads, g[:, 0:3] = 0
                # (the kk=0 edge feature is identically zero).
                if ablate in (None, "xs", "dsf", "dm"):
                    for _ in range(10):
                        dm0 = wpool.tile([P, 16], f32, tag="dm")
                        nc.vector.memset(dm0[:, 12:16], -3.0e38)
                if ablate in (None, "xs"):
                    for _ in range(10):
                        g0 = wpool.tile([P, 18], f32r, tag="g")
                        nc.vector.memset(g0[:, 0:3].bitcast(u32), 0)

                m32s, cands, difs, sqs, scans_out, sels, prodss = {}, {}, {}, {}, {}, {}, {}

                def stage1a(t):
                    """dist matmuls -> ACT evict (S, |D|) -> CCE pair key."""
                    m32 = mpool.tile([P, H], f32, tag="m32")
                    tt = dpool.tile([P, H], f32, tag="dist")
                    for h in range(4):
                        ps = ppool.tile([P, 1024], f32, tag="ps")
                        for j in range(2):
                            col0 = h * 1024 + j * 512
                            nc.tensor.matmul(
                                out=ps[:, j * 512:(j + 1) * 512],
                                lhsT=lt_sb[:, t * P:(t + 1) * P],
                                rhs=rt_sb[:, col0:col0 + 512],
                                start=True,
                                stop=True,
                            )
                        dst = (m32[:, h * 1024:(h + 1) * 1024] if h < 2
                               else tt[:, (h - 2) * 1024:(h - 1) * 1024])
                        nc.scalar.activation(
                            out=dst,
                            in_=ps[:],
                            func=(Copy if h < 2
                                  else mybir.ActivationFunctionType.Abs),
                            scale=0.5,
                        )
                    if ablate == "dist":
                        if t == NT - 1:
                            res0 = wpool.tile([P, 1], f32, tag="res0")
                            nc.vector.reduce_max(
                                out=res0[:], in_=tt[:],
                                axis=mybir.AxisListType.X)
                            nc.sync.dma_start(out=out_p[0:64, 0:1],
                                              in_=res0[0:64, :])
                        return
                    nc.gpsimd.dma_start(out=m32[:, 0:Q], in_=tt[:, 0:Q],
                                        accum_op=mybir.AluOpType.add)
                    nc.gpsimd.dma_start(out=m32[:, Q:H], in_=tt[:, Q:H],
                                        accum_op=mybir.AluOpType.add)
                    if ablate == "m":
                        if t == NT - 1:
                            res0 = wpool.tile([P, 1], f32, tag="res0")
                            nc.vector.reduce_max(
                                out=res0[:], in_=m32[:],
                                axis=mybir.AxisListType.X)
                            nc.sync.dma_start(out=out_p[0:64, 0:1],
                                              in_=res0[0:64, :])
                        return
                    m32s[t] = m32

                def stage1b(t):
                    """quad fold -> top-8 scan -> candidate gathers."""
                    m32 = m32s.pop(t)
                    qk = mpool.tile([P, Q], f32, tag="qk")
                    nc.vector.tensor_max(out=qk[:], in0=m32[:, 0:Q],
                                         in1=m32[:, Q:H])
                    if ablate == "q":
                        if t == NT - 1:
                            res0 = wpool.tile([P, 1], f32, tag="res0")
                            nc.vector.reduce_max(
                                out=res0[:], in_=qk[:],
                                axis=mybir.AxisListType.X)
                            nc.sync.dma_start(out=out_p[0:64, 0:1],
                                              in_=res0[0:64, :])
                        return
                    maxv = wpool.tile([P, 8], f32, tag="maxv")
                    nc.vector.max(out=maxv[:], in_=qk[:])
                    idxs = wpool.tile([P, 8], u32, tag="idxs")
                    nc.vector.max_index(out=idxs[:], in_max=maxv[:],
                                        in_values=qk[:])
                    if ablate == "idx":
                        if t == NT - 1:
                            idf = wpool.tile([P, 3], f32, tag="idf")
                            nc.vector.tensor_copy(out=idf[:], in_=idxs[:, 0:3])
                            nc.sync.dma_start(out=out_p[0:64, 0:3],
                                              in_=idf[0:64, :])
                        return
                    candp = wpool.tile([P, 3, 24], f32, tag="cand")
                    sp0 = 128 * (t % 8)
                    nc.sync.dma_start(out=candp[:, 0, :],
                                      in_=xq[sp0:sp0 + 128, :])
                    for s in range(1, 3):
                        nc.gpsimd.indirect_dma_start(
                            out=candp[:, s, :],
                            out_offset=None,
                            in_=xq[:],
                            in_offset=bass.IndirectOffsetOnAxis(
                                ap=idxs[:, s:s + 1], axis=0
                            ),
                        )
                    if ablate == "cand":
                        if t == NT - 1:
                            nc.sync.dma_start(out=out_p[0:64, 0:3],
                                              in_=candp[0:64, 0, 12:15])
                        return
                    cands[t] = candp

                def stage2a(t):
                    """refine distances: dif (gpsimd)."""
                    candp = cands[t]
                    cand_d = candp[:, :, 0:12].rearrange(
                        "p a (m c) -> p a m c", m=4)          # (P, 3, 4, 3)
                    dif = wpool.tile([P, 3, 4, 3], f32, tag="dif")
                    nc.gpsimd.tensor_sub(
                        out=dif[:], in0=cand_d,
                        in1=yall[:, t:t + 1, :].rearrange(
                            "p (a o) c -> p a o c", o=1
                        ).to_broadcast([P, 3, 4, 3]),
                    )
                    difs[t] = dif

                def stage2b(t):
                    """refine distances: square (ACT)."""
                    dif = difs.pop(t)
                    sq = wpool.tile([P, 3, 4, 3], f32, tag="sq")
                    nc.scalar.square(out=sq[:], in_=dif[:])
                    sqs[t] = sq

                def stage2c(t):
                    """refine reduce + top-3-of-12 scan (DVE)."""
                    sq = sqs.pop(t)
                    dm = wpool.tile([P, 16], f32, tag="dm")
                    nc.vector.tensor_reduce(
                        out=dm[:, 0:12].rearrange("p (a m) -> p a m", m=4),
                        in_=sq[:], axis=mybir.AxisListType.X,
                        op=mybir.AluOpType.add, negate=True,
                    )
                    if ablate == "dm":
                        if t == NT - 1:
                            nc.sync.dma_start(out=out_p[0:64, 0:3],
                                              in_=dm[0:64, 0:3])
                        return
                    dv = wpool.tile([P, 8], f32, tag="dv")
                    nc.vector.max(out=dv[:], in_=dm[:])
                    dslots = wpool.tile([P, 8], u32, tag="dslots")
                    nc.vector.max_index(out=dslots[:], in_max=dv[:],
                                        in_values=dm[:])
                    scans_out[t] = dslots

                def stage2d(t):
                    """one-hot select of the k=1,2 neighbors (gpsimd)."""
                    dslots = scans_out.pop(t)
                    candp = cands.pop(t)
                    dsf = wpool.tile([P, 2], f32, tag="dsf")
                    nc.gpsimd.tensor_copy(out=dsf[:], in_=dslots[:, 1:3])
                    if ablate == "dsf":
                        if t == NT - 1:
                            dsl = wpool.tile([P, 3], f32, tag="dsl")
                            nc.vector.tensor_copy(out=dsl[:],
                                                  in_=dslots[:, 0:3])
                            nc.sync.dma_start(out=out_p[0:64, 0:3],
                                              in_=dsl[0:64, :])
                        return
                    oh = wpool.tile([P, 2, 16], f32, tag="oh")
                    for k in range(2):
                        nc.gpsimd.tensor_scalar(
                            out=oh[:, k, :], in0=iota16[:],
                            scalar1=dsf[:, k:k + 1], scalar2=None,
                            op0=mybir.AluOpType.is_equal,
                        )
                    cand_x = candp[:, :, 12:24].rearrange(
                        "p a (m c) -> p a m c", m=4)          # (P, 3, 4, 3)
                    # edge vectors d = x_cand - x_n computed exactly in fp32
                    # BEFORE the one-hot sum (keeps fp32r rounding relative
                    # to the small d, not the O(1) coords).
                    cand_xd = wpool.tile([P, 3, 4, 3], f32, tag="cand_xd")
                    nc.gpsimd.tensor_sub(
                        out=cand_xd[:], in0=cand_x,
                        in1=xall[:, t:t + 1, :].rearrange(
                            "p (a o) c -> p a o c", o=1
                        ).to_broadcast([P, 3, 4, 3]),
                    )
                    prods = wpool.tile([P, 2, 3, 4, 3], f32, tag="prods")
                    for k in range(2):
                        nc.gpsimd.tensor_mul(
                            out=prods[:, k],
                            in0=cand_xd[:],
                            in1=oh[:, k, 0:12].rearrange(
                                "p (a m o) -> p a m o", m=4, o=1
                            ).to_broadcast([P, 3, 4, 3]),
                        )
                    prodss[t] = prods

                def stage2e(t):
                    """g assembly -> stream store -> gated conv chunks."""
                    prods = prodss.pop(t)
                    g = wpool.tile([P, 18], f32r, tag="g")
                    g3 = g[:].rearrange("p (a b) -> p a b", a=3)
                    nc.scalar.activation(
                        out=g3[:, :, 3:6],
                        in_=xall[:, t:t + 1, :].to_broadcast([P, 3, 3]),
                        func=Copy,
                    )
                    with nc.allow_low_precision(
                            reason="f32r SBUF stores full fp32 bits"):
                        nc.vector.tensor_reduce(
                            out=g3[:, 1:3, 0:3],
                            in_=prods[:].rearrange("p k a m c -> p k c (a m)"),
                            axis=mybir.AxisListType.X,
                            op=mybir.AluOpType.add,
                        )
                    if ablate == "xs":
                        if t == NT - 1:
                            nc.sync.dma_start(out=out_p[0:64, 0:3],
                                              in_=g[0:64, 6:9].bitcast(f32))
                        return
                    nc.sync.dma_start(
                        out=g_sep[t * P:(t + 1) * P, :], in_=g[:])
                    for m in range(12):
                        c_hi = 2 * m + 1
                        n_max = (5 * 12288 + 512 * (c_hi + 1) - 1) // 18
                        gate = n_max // P
                        if gate != t:
                            continue
                        psc = cppool.tile([64, 2, 512], f32, tag="pst")
                        for half in range(2):
                            c = 2 * m + half
                            g24c = wpool.tile([6, 512], f32r, tag="g24c")
                            hlv = g_sep[:].flatten().rearrange(
                                "(x b) -> x b", x=6)[:, c * 512:(c + 1) * 512]
                            nc.sync.dma_start(out=g24c[:], in_=hlv)
                            nc.tensor.matmul(
                                out=psc[:, half, :],
                                lhsT=w6_sb[:],
                                rhs=g24c[:],
                                start=True,
                                stop=True,
                            )
                        nc.vector.reduce_max(
                            out=partial2a[:, 2 * m:2 * m + 2],
                            in_=psc[:],
                            axis=mybir.AxisListType.X,
                        )

                # depth-6 software pipeline: each cross-engine dependency is
                # at least one iteration old, so no engine queue head ever
                # waits on work emitted in the same iteration. Deepest stage
                # first so ready work sits at each queue head.
                stages = [(6, stage2e), (5, stage2d), (4, stage2c),
                          (3, stage2b), (2, stage2a), (1, stage1b),
                          (0, stage1a)]
                cut = {"dist": 0, "m": 0, "q": 1, "max": 1, "idx": 1,
                       "cand": 1, "dm": 4, "dsf": 5, "xs": 6}.get(ablate, 6)
                for it in range(NT + 6):
                    for lag, fn in stages:
                        if lag > cut:
                            continue
                        tt_ = it - lag
                        if 0 <= tt_ < NT:
                            fn(tt_)

              if ablate is None:
                res = wpool.tile([64, 3], f32, tag="res")
                nc.vector.reduce_max(
                    out=res[:],
                    in_=partial2a[:].rearrange("p (k g) -> p k g", k=3),
                    axis=mybir.AxisListType.X,
                )
                nc.sync.dma_start(out=out_p[:], in_=res[:])
    nc.compile()
    return nc


def _get_nc():
    global _compiled
    if _compiled is None:
        _compiled = _build()
    return _compiled


def _split_fp32r(a):
    """Exact split a = hi + lo with both pieces having <= 12 significant bits."""
    a = np.ascontiguousarray(a, np.float32)
    hi = (a.view(np.uint32) & np.uint32(0xFFFFF000)).view(np.float32)
    lo = (a - hi).astype(np.float32)
    return hi, lo


def _make_in_maps(x, w1):
    """x: (B, 4096, 3) float32, w1: (64, 6) -> per-core input dicts.

    lt rows (K=16):
      k0-11 : 2*y_piece[n] (pieces h,h,l,l x c=0..2)
      k12-13: xx_n pieces
      k14-15: -1
    rt = [rtS | rtD] (16, 2048+2048): the S columns make the matmul emit
    S[n,j] = dist[n,j] + dist[n,j+2048]; the D columns emit
    D[n,j] = dist[n,j] - dist[n,j+2048]. All rhs entries are re-split to
    <=12 significant bits so every fp32r product stays exact.

    xquad rows (1024, 24): [Y_j Y_j+1024 Y_j+2048 Y_j+3072 | x_... same].
    w3 (3, 192): [w1x.T | w1d.T | (w1x-w1d).T] for the folded conv.
    """
    H = N // 2
    Q = N // 4
    w1 = np.ascontiguousarray(w1, np.float32)
    w6 = np.ascontiguousarray(w1.T)
    iota16 = np.ascontiguousarray(
        np.tile(np.arange(16, dtype=np.float32), (P, 1)))
    in_maps = []
    for b in range(B):
        xb = np.ascontiguousarray(x[b], dtype=np.float32)       # (4096, 3)
        y = xb.reshape(3, N)                                     # flat view
        xx = np.sum(y * y, axis=0, dtype=np.float32)             # (4096,)
        yh, yl = _split_fp32r(y)
        xh, xl = _split_fp32r(xx)
        lt = np.empty((16, N), np.float32)
        for i, la in enumerate([yh, yh, yl, yl]):
            lt[3 * i:3 * i + 3] = 2.0 * la
        lt[12], lt[13] = xh, xl
        lt[14], lt[15] = -1.0, -1.0

        s = (y[:, :H] + y[:, H:]).astype(np.float32)
        dd = (y[:, :H] - y[:, H:]).astype(np.float32)
        sh, sl = _split_fp32r(s)
        dh, dl = _split_fp32r(dd)
        xxs = (xx[:H] + xx[H:]).astype(np.float32)
        xxd = (xx[:H] - xx[H:]).astype(np.float32)
        xxs_h, xxs_l = _split_fp32r(xxs)
        xxd_h, xxd_l = _split_fp32r(xxd)
        rt = np.empty((16, N), np.float32)
        for i, ra in enumerate([sh, sl, sh, sl]):
            rt[3 * i:3 * i + 3, :H] = ra
        for i, ra in enumerate([dh, dl, dh, dl]):
            rt[3 * i:3 * i + 3, H:] = ra
        rt[12:14, :H] = -2.0
        rt[14, :H], rt[15, :H] = xxs_h, xxs_l
        rt[12:14, H:] = 0.0
        rt[14, H:], rt[15, H:] = xxd_h, xxd_l

        Y = np.ascontiguousarray(y.T, np.float32)              # (4096, 3) col coords
        xquad = np.ascontiguousarray(np.concatenate(
            [Y[0:Q], Y[Q:2 * Q], Y[2 * Q:3 * Q], Y[3 * Q:4 * Q],
             xb[0:Q], xb[Q:2 * Q], xb[2 * Q:3 * Q], xb[3 * Q:4 * Q]],
            axis=1), np.float32)
        in_maps.append({"lt": lt, "rt": rt, "xrows": xb, "ycols": Y,
                        "xquad": xquad, "iota16": iota16, "w6": w6})
    return in_maps


def run_device(x, w1, trace=False):
    """Run the per-sample stage-1 kernel on 8 cores.

    Returns (seg_max (B, 64, 3) float32, exec_time_ns or None).
    """
    from concourse.bass_utils import run_bass_kernel_spmd

    nc = _get_nc()
    in_maps = _make_in_maps(x, w1)
    core_ids = list(range(B))
    r = run_bass_kernel_spmd(nc, in_maps, core_ids, trace=trace)
    seg = np.stack([np.asarray(r.results[i]["out"]) for i in range(B)])
    return seg, r.exec_time_ns


# ---------------- host-side downstream (mirrors reference.py in fp32) -------

def _topk_idx(dist, k):
    # lax.top_k: descending values, ties -> lowest index first
    return np.argsort(-dist, axis=-1, kind="stable")[..., :k]


def _get_graph_feature_np(x, k):
    Bb, Nn = x.shape[0], x.shape[1]
    x = x.reshape(Bb, -1, Nn)
    C = x.shape[1]
    inner = (-2.0 * np.einsum("bcn,bcm->bnm", x, x)).astype(np.float32)
    xx = np.sum(x * x, axis=1, keepdims=True, dtype=np.float32)
    dist = -xx - inner - np.swapaxes(xx, 1, 2)
    idx = _topk_idx(dist, k)
    flat = x.reshape(Bb * Nn, C)
    idx_full = (idx + (np.arange(Bb) * Nn)[:, None, None]).reshape(-1)
    feature = flat[idx_full].reshape(Bb, Nn, k, C)
    xc = x.reshape(Bb, Nn, 1, C)
    feature = np.concatenate(
        [feature - xc, np.broadcast_to(xc, (Bb, Nn, k, C))], axis=3
    )
    return feature.reshape(Bb, 2 * C, k, Nn)


def _bn(h, g, b, m, v, axis):
    shape = [1] * h.ndim
    shape[axis] = -1
    inv = (g.reshape(shape) / np.sqrt(v.reshape(shape) + EPS)).astype(np.float32)
    return ((h - m.reshape(shape)) * inv + b.reshape(shape)).astype(np.float32)


def _leaky(x):
    return np.where(x >= 0, x, np.float32(0.2) * x).astype(np.float32)


def _softmax(z):
    z = z - np.max(z, axis=1, keepdims=True)
    e = np.exp(z)
    return (e / np.sum(e, axis=1, keepdims=True)).astype(np.float32)


def kernel(x, w1, wA, bA, wB, bB, wC, bC,
           bn1_g, bn1_b, bn1_m, bn1_v,
           bnA_g, bnA_b, bnA_m, bnA_v,
           bnB_g, bnB_b, bnB_m, bnB_v):
    x = np.asarray(x, np.float32)
    w1 = np.asarray(w1, np.float32)

    seg, _ = run_device(x, w1)                               # (B, 64, 3) raw maxima
    x1 = _leaky(_bn(seg, np.asarray(bn1_g, np.float32), np.asarray(bn1_b, np.float32),
                    np.asarray(bn1_m, np.float32), np.asarray(bn1_v, np.float32), 1))
    x2 = np.max(_get_graph_feature_np(x1, K), axis=-1)
    x3 = np.max(_get_graph_feature_np(x2, K), axis=-1)
    x4 = np.max(_get_graph_feature_np(x3, K), axis=-1)
    xc = np.concatenate([x1, x2, x3, x4], axis=1)            # (B, 82, 3)
    p = np.max(xc, axis=-1)                                  # (B, 82)
    h = _leaky(_bn(p @ np.asarray(wA, np.float32).T + np.asarray(bA, np.float32),
                   np.asarray(bnA_g, np.float32), np.asarray(bnA_b, np.float32),
                   np.asarray(bnA_m, np.float32), np.asarray(bnA_v, np.float32), 1))
    h = _leaky(_bn(h @ np.asarray(wB, np.float32).T + np.asarray(bB, np.float32),
                   np.asarray(bnB_g, np.float32), np.asarray(bnB_b, np.float32),
                   np.asarray(bnB_m, np.float32), np.asarray(bnB_v, np.float32), 1))
    return _softmax(h @ np.asarray(wC, np.float32).T + np.asarray(bC, np.float32))


# revision 19
# speedup vs baseline: 1.2184x; 1.2184x over previous
"""DGCNN forward kernel for Trainium2 (8 NeuronCores, data-parallel over batch).

Contract: kernel(**inputs) takes the FULL unsharded inputs (keyed as in
setup_inputs()) and returns the FULL (8, 3) float32 output.

Strategy
--------
B = 8 samples -> 1 sample per NeuronCore (pure data parallel; the tiny weights
are replicated). Per sample, the dominant work is stage 1 of the DGCNN:

  y     = x.reshape(3, 4096)            (flat view, matches the torch .view)
  dist  = (2*y^T y - xx_n) - xx_m       (4096 x 4096)
  idx   = top-3 columns per row         (includes self)
  x1    = leaky(bn1(max_n w1 @ [x[idx]-x[n]; x[n]]))   -> (64, 3)

Everything through the segment-max runs on device; bn1+leaky (monotone, so it
commutes with the max bit-exactly) and the later KNN stages on 64/6-point
clouds + the tiny MLP (~2 KFLOP total, <0.01% of the FLOPs) run on host in
float32, mirroring the reference ops exactly.

Distance matmul: operands are split exactly into hi+lo FP32R pieces (<=12
significant bits each, so every product is exact) and folded into one K=16
FP32R matmul per 512-column chunk, accumulated in PSUM in the reference's
rounding order.

Selection uses a QUAD-REDUCTION: columns {j, j+1024, j+2048, j+3072} form a
group. The PE emits S = distA+distB (cols 0:2048) and D = distA-distB (cols
2048:4096) for the (j, j+2048) pairs; ACT evicts 0.5*S (Copy) and 0.5*|D|
(Abs); one gpsimd CCE DMA (accum add) forms the pair key
m[j] = max(distA, distB); one DVE tensor_max folds pairs (j, j+1024) into
quad keys q[j] = max of the 4 group members. The DVE top-8/index scans then
run on 1024 keys instead of 4096 (the scans are the DVE bottleneck; they are
elem-count-bound at ~1/cycle regardless of dtype).

Exactness: the top-3 quads by q provably contain {self, nn1, nn2} (a quad
outranks nn2's quad only if it holds self or nn1). All 4 members of the top-3
quads are gathered as 96B rows [Ycol x4 | xrow x4] (SWDGE); the refine
recomputes exact squared distances from the FLAT-VIEW COLUMN coords Y (the
reference's distance space) while features use the flat-view ROW coords x
(the reference's gather space). The self candidate has d = -0.0 exactly, so
it is always refine-rank 0; ranks 1,2 are the k=1,2 neighbors, selected by a
one-hot sum over 12 slots. Numpy-validated: 0/32768 neighbor mismatches vs
the fp32 reference on the test data.

The reference conv contracts w1 against a FLAT .view() of the n-major
edge-feature stream (g.reshape(6, 12288) crosses point boundaries), so the
per-tile g rows ([d_kk(3) | x_n(3)] x 3, with d exactly fp32-subtracted)
round-trip through a DRAM scratch buffer whose flat re-view feeds plain
fp32 K=6 matmuls (exact; the PE has slack). Chunk maxima respect the k'
quarters and fold into the (64, 3) segment maxima at the end.

Per 128-row tile, software-pipelined (stage2 lags stage1 by LAG=3 tiles):
  PE    : 8 K=16 FP32R dist matmuls -> PSUM (S|D), fp32 conv matmuls
  ACT   : PSUM evict (Copy 0.5*S, Abs 0.5*D), refine square, g x-blocks
  DMA   : gpsimd CCE add m += tt; SWDGE candidate gathers; g stream store+load
  DVE   : quad tensor_max + InstMax/InstMaxIndex on q (1024) + refine scans
          + conv reduce_max
  GPSIMD: gathers + refine smalls (sub, one-hot select)
"""

import numpy as np

N = 4096
P = 128
NT = N // P           # 32 row tiles
B = 8
EPS = 1e-5
K = 3

_compiled = None


def _build(reps=1, ablate=None):
    # ablate: None=full, "dist"=PE+ACT only, "m"=+gpsimd pair-max,
    #         "q"=+DVE quad-max, "max"=+InstMax, "idx"=+InstMaxIndex,
    #         "cand"=+gathers, "dm"=+refine dist, "dsf"=+refine scans,
    #         "xs"=+one-hot select (no conv/transpose)
    import contextlib
    HCDBG = globals().get('HCDBG_COL', 128)

    import concourse.bass as bass
    import concourse.mybir as mybir
    from concourse import bacc
    from concourse.tile import TileContext

    f32 = mybir.dt.float32
    f32r = mybir.dt.float32r
    u32 = mybir.dt.uint32
    Copy = mybir.ActivationFunctionType.Copy
    H = N // 2
    Q = N // 4

    nc = bacc.Bacc(
        "TRN2", target_bir_lowering=False, debug=False, num_devices=B
    )
    lt = nc.declare_dram_parameter("lt", [16, N], f32r, isOutput=False)
    rt = nc.declare_dram_parameter("rt", [16, N], f32r, isOutput=False)
    xr = nc.declare_dram_parameter("xrows", [N, 3], f32, isOutput=False)
    xq = nc.declare_dram_parameter("xquad", [Q, 24], f32, isOutput=False)
    yc = nc.declare_dram_parameter("ycols", [N, 3], f32, isOutput=False)
    io16 = nc.declare_dram_parameter("iota16", [P, 16], f32, isOutput=False)
    w6p = nc.declare_dram_parameter("w6", [6, 64], f32r, isOutput=False)
    out_p = nc.declare_dram_parameter("out", [64, 3], f32, isOutput=True)

    g_sep = nc.dram_tensor("g_scratch", [N, 18], f32r)

    with TileContext(nc) as tc:
        with (
            tc.tile_pool(name="const", bufs=1) as cpool,
            tc.tile_pool(name="dist", bufs=4) as dpool,
            tc.tile_pool(name="mkey", bufs=6) as mpool,
            tc.tile_pool(name="work", bufs=10) as wpool,
        ):
            lt_sb = cpool.tile([16, N], f32r)
            nc.sync.dma_start(out=lt_sb[:, 0:256], in_=lt[:, 0:256])
            nc.sync.dma_start(out=lt_sb[:, 256:N], in_=lt[:, 256:N])
            rt_sb = cpool.tile([16, N], f32r)
            for cchunk in range(4):
                nc.sync.dma_start(
                    out=rt_sb[:, cchunk * 1024:(cchunk + 1) * 1024],
                    in_=rt[:, cchunk * 1024:(cchunk + 1) * 1024],
                )
            w6_sb = cpool.tile([6, 64], f32r)
            nc.sync.dma_start(out=w6_sb[:], in_=w6p[:])
            xall = cpool.tile([P, NT, 3], f32)
            nc.sync.dma_start(
                out=xall[:],
                in_=xr[:].rearrange("(t p) c -> p t c", p=P),
            )
            yall = cpool.tile([P, NT, 3], f32)
            nc.sync.dma_start(
                out=yall[:],
                in_=yc[:].rearrange("(t p) c -> p t c", p=P),
            )
            iota16 = cpool.tile([P, 16], f32)
            nc.sync.dma_start(out=iota16[:], in_=io16[:])

            loop_cm = tc.For_i(0, reps, 1) if reps > 1 else contextlib.nullcontext()
            with loop_cm:
              with tc.tile_pool(name="psum_d", bufs=3, space="PSUM") as ppool, \
                   tc.tile_pool(name="psum_c", bufs=1, space="PSUM") as cppool:
                partial2a = wpool.tile([64, 24], f32, tag="partial2a")
                # pre-initialize ring-buffer slots the steady-state loop
                # never writes: dm[:, 12:16] = -inf pads, g[:, 0:3] = 0
                # (the kk=0 edge feature is identically zero).
                if ablate in (None, "xs", "dsf", "dm"):
                    for _ in range(10):
                        dm0 = wpool.tile([P, 16], f32, tag="dm")
                        nc.vector.memset(dm0[:, 12:16], -3.0e38)
                if ablate in (None, "xs"):
                    for _ in range(10):
                        g0 = wpool.tile([P, 18], f32r, tag="g")
                        nc.vector.memset(g0[:, 0:3].bitcast(u32), 0)

                m32s, cands, difs, sqs, scans_out, sels, prodss = {}, {}, {}, {}, {}, {}, {}

                def stage1a(t):
                    """dist matmuls -> ACT evict (S, |D|) -> CCE pair key."""
                    m32 = mpool.tile([P, H], f32, tag="m32")
                    tt = dpool.tile([P, H], f32, tag="dist")
                    for h in range(4):
                        ps = ppool.tile([P, 1024], f32, tag="ps")
                        for j in range(2):
                            col0 = h * 1024 + j * 512
                            nc.tensor.matmul(
                                out=ps[:, j * 512:(j + 1) * 512],
                                lhsT=lt_sb[:, t * P:(t + 1) * P],
                                rhs=rt_sb[:, col0:col0 + 512],
                                start=True,
                                stop=True,
                            )
                        dst = (m32[:, h * 1024:(h + 1) * 1024] if h < 2
                               else tt[:, (h - 2) * 1024:(h - 1) * 1024])
                        nc.scalar.activation(
                            out=dst,
                            in_=ps[:],
                            func=(Copy if h < 2
                                  else mybir.ActivationFunctionType.Abs),
                            scale=0.5,
                        )
                    if ablate == "dist":
                        if t == NT - 1:
                            res0 = wpool.tile([P, 1], f32, tag="res0")
                            nc.vector.reduce_max(
                                out=res0[:], in_=tt[:],
                                axis=mybir.AxisListType.X)
                            nc.sync.dma_start(out=out_p[0:64, 0:1],
                                              in_=res0[0:64, :])
                        return
                    nc.gpsimd.dma_start(out=m32[:], in_=tt[:],
                                        accum_op=mybir.AluOpType.add)
                    if ablate == "m":
                        if t == NT - 1:
                            res0 = wpool.tile([P, 1], f32, tag="res0")
                            nc.vector.reduce_max(
                                out=res0[:], in_=m32[:],
                                axis=mybir.AxisListType.X)
                            nc.sync.dma_start(out=out_p[0:64, 0:1],
                                              in_=res0[0:64, :])
                        return
                    m32s[t] = m32

                def stage1b(t):
                    """quad fold -> top-8 scan -> candidate gathers."""
                    m32 = m32s.pop(t)
                    qk = mpool.tile([P, Q], f32, tag="qk")
                    nc.vector.tensor_max(out=qk[:], in0=m32[:, 0:Q],
                                         in1=m32[:, Q:H])
                    if ablate == "q":
                        if t == NT - 1:
                            res0 = wpool.tile([P, 1], f32, tag="res0")
                            nc.vector.reduce_max(
                                out=res0[:], in_=qk[:],
                                axis=mybir.AxisListType.X)
                            nc.sync.dma_start(out=out_p[0:64, 0:1],
                                              in_=res0[0:64, :])
                        return
                    maxv = wpool.tile([P, 8], f32, tag="maxv")
                    nc.vector.max(out=maxv[:], in_=qk[:])
                    idxs = wpool.tile([P, 8], u32, tag="idxs")
                    nc.vector.max_index(out=idxs[:], in_max=maxv[:],
                                        in_values=qk[:])
                    if ablate == "idx":
                        if t == NT - 1:
                            idf = wpool.tile([P, 3], f32, tag="idf")
                            nc.vector.tensor_copy(out=idf[:], in_=idxs[:, 0:3])
                            nc.sync.dma_start(out=out_p[0:64, 0:3],
                                              in_=idf[0:64, :])
                        return
                    candp = wpool.tile([P, 3, 24], f32, tag="cand")
                    sp0 = 128 * (t % 8)
                    nc.sync.dma_start(out=candp[:, 0, :],
                                      in_=xq[sp0:sp0 + 128, :])
                    for s in range(1, 3):
                        nc.gpsimd.indirect_dma_start(
                            out=candp[:, s, :],
                            out_offset=None,
                            in_=xq[:],
                            in_offset=bass.IndirectOffsetOnAxis(
                                ap=idxs[:, s:s + 1], axis=0
                            ),
                        )
                    if ablate == "cand":
                        if t == NT - 1:
                            nc.sync.dma_start(out=out_p[0:64, 0:3],
                                              in_=candp[0:64, 0, 12:15])
                        return
                    cands[t] = candp

                def stage2a(t):
                    """refine distances: dif (gpsimd)."""
                    candp = cands[t]
                    cand_d = candp[:, :, 0:12].rearrange(
                        "p a (m c) -> p a m c", m=4)          # (P, 3, 4, 3)
                    dif = wpool.tile([P, 3, 4, 3], f32, tag="dif")
                    nc.gpsimd.tensor_sub(
                        out=dif[:], in0=cand_d,
                        in1=yall[:, t:t + 1, :].rearrange(
                            "p (a o) c -> p a o c", o=1
                        ).to_broadcast([P, 3, 4, 3]),
                    )
                    difs[t] = dif

                def stage2b(t):
                    """refine distances: square (ACT)."""
                    dif = difs.pop(t)
                    sq = wpool.tile([P, 3, 4, 3], f32, tag="sq")
                    nc.scalar.square(out=sq[:], in_=dif[:])
                    sqs[t] = sq

                def stage2c(t):
                    """refine reduce + top-3-of-12 scan (DVE)."""
                    sq = sqs.pop(t)
                    dm = wpool.tile([P, 16], f32, tag="dm")
                    nc.vector.tensor_reduce(
                        out=dm[:, 0:12].rearrange("p (a m) -> p a m", m=4),
                        in_=sq[:], axis=mybir.AxisListType.X,
                        op=mybir.AluOpType.add, negate=True,
                    )
                    if ablate == "dm":
                        if t == NT - 1:
                            nc.sync.dma_start(out=out_p[0:64, 0:3],
                                              in_=dm[0:64, 0:3])
                        return
                    dv = wpool.tile([P, 8], f32, tag="dv")
                    nc.vector.max(out=dv[:], in_=dm[:])
                    dslots = wpool.tile([P, 8], u32, tag="dslots")
                    nc.vector.max_index(out=dslots[:], in_max=dv[:],
                                        in_values=dm[:])
                    scans_out[t] = dslots

                def stage2d(t):
                    """one-hot select of the k=1,2 neighbors (gpsimd)."""
                    dslots = scans_out.pop(t)
                    candp = cands.pop(t)
                    dsf = wpool.tile([P, 2], f32, tag="dsf")
                    nc.gpsimd.tensor_copy(out=dsf[:], in_=dslots[:, 1:3])
                    if ablate == "dsf":
                        if t == NT - 1:
                            dsl = wpool.tile([P, 3], f32, tag="dsl")
                            nc.vector.tensor_copy(out=dsl[:],
                                                  in_=dslots[:, 0:3])
                            nc.sync.dma_start(out=out_p[0:64, 0:3],
                                              in_=dsl[0:64, :])
                        return
                    oh = wpool.tile([P, 2, 16], f32, tag="oh")
                    for k in range(2):
                        nc.gpsimd.tensor_scalar(
                            out=oh[:, k, :], in0=iota16[:],
                            scalar1=dsf[:, k:k + 1], scalar2=None,
                            op0=mybir.AluOpType.is_equal,
                        )
                    cand_x = candp[:, :, 12:24].rearrange(
                        "p a (m c) -> p a m c", m=4)          # (P, 3, 4, 3)
                    # edge vectors d = x_cand - x_n computed exactly in fp32
                    # BEFORE the one-hot sum (keeps fp32r rounding relative
                    # to the small d, not the O(1) coords).
                    cand_xd = wpool.tile([P, 3, 4, 3], f32, tag="cand_xd")
                    nc.gpsimd.tensor_sub(
                        out=cand_xd[:], in0=cand_x,
                        in1=xall[:, t:t + 1, :].rearrange(
                            "p (a o) c -> p a o c", o=1
                        ).to_broadcast([P, 3, 4, 3]),
                    )
                    prods = wpool.tile([P, 2, 3, 4, 3], f32, tag="prods")
                    for k in range(2):
                        nc.gpsimd.tensor_mul(
                            out=prods[:, k],
                            in0=cand_xd[:],
                            in1=oh[:, k, 0:12].rearrange(
                                "p (a m o) -> p a m o", m=4, o=1
                            ).to_broadcast([P, 3, 4, 3]),
                        )
                    prodss[t] = prods

                def stage2e(t):
                    """g assembly -> stream store -> gated conv chunks."""
                    prods = prodss.pop(t)
                    g = wpool.tile([P, 18], f32r, tag="g")
                    g3 = g[:].rearrange("p (a b) -> p a b", a=3)
                    nc.scalar.activation(
                        out=g3[:, :, 3:6],
                        in_=xall[:, t:t + 1, :].to_broadcast([P, 3, 3]),
                        func=Copy,
                    )
                    with nc.allow_low_precision(
                            reason="f32r SBUF stores full fp32 bits"):
                        nc.vector.tensor_reduce(
                            out=g3[:, 1:3, 0:3],
                            in_=prods[:].rearrange("p k a m c -> p k c (a m)"),
                            axis=mybir.AxisListType.X,
                            op=mybir.AluOpType.add,
                        )
                    if ablate == "xs":
                        if t == NT - 1:
                            nc.sync.dma_start(out=out_p[0:64, 0:3],
                                              in_=g[0:64, 6:9].bitcast(f32))
                        return
                    nc.sync.dma_start(
                        out=g_sep[t * P:(t + 1) * P, :], in_=g[:])
                    for m in range(12):
                        c_hi = 2 * m + 1
                        n_max = (5 * 12288 + 512 * (c_hi + 1) - 1) // 18
                        gate = n_max // P
                        if gate != t:
                            continue
                        psc = cppool.tile([64, 2, 512], f32, tag="pst")
                        for half in range(2):
                            c = 2 * m + half
                            g24c = wpool.tile([6, 512], f32r, tag="g24c")
                            hlv = g_sep[:].flatten().rearrange(
                                "(x b) -> x b", x=6)[:, c * 512:(c + 1) * 512]
                            nc.sync.dma_start(out=g24c[:], in_=hlv)
                            nc.tensor.matmul(
                                out=psc[:, half, :],
                                lhsT=w6_sb[:],
                                rhs=g24c[:],
                                start=True,
                                stop=True,
                            )
                        nc.vector.reduce_max(
                            out=partial2a[:, 2 * m:2 * m + 2],
                            in_=psc[:],
                            axis=mybir.AxisListType.X,
                        )

                # depth-6 software pipeline: each cross-engine dependency is
                # at least one iteration old, so no engine queue head ever
                # waits on work emitted in the same iteration. Deepest stage
                # first so ready work sits at each queue head.
                stages = [(6, stage2e), (5, stage2d), (4, stage2c),
                          (3, stage2b), (2, stage2a), (1, stage1b),
                          (0, stage1a)]
                cut = {"dist": 0, "m": 0, "q": 1, "max": 1, "idx": 1,
                       "cand": 1, "dm": 4, "dsf": 5, "xs": 6}.get(ablate, 6)
                for it in range(NT + 6):
                    for lag, fn in stages:
                        if lag > cut:
                            continue
                        tt_ = it - lag
                        if 0 <= tt_ < NT:
                            fn(tt_)

              if ablate is None:
                res = wpool.tile([64, 3], f32, tag="res")
                nc.vector.reduce_max(
                    out=res[:],
                    in_=partial2a[:].rearrange("p (k g) -> p k g", k=3),
                    axis=mybir.AxisListType.X,
                )
                nc.sync.dma_start(out=out_p[:], in_=res[:])
    nc.compile()
    return nc


def _get_nc():
    global _compiled
    if _compiled is None:
        _compiled = _build()
    return _compiled


def _split_fp32r(a):
    """Exact split a = hi + lo with both pieces having <= 12 significant bits."""
    a = np.ascontiguousarray(a, np.float32)
    hi = (a.view(np.uint32) & np.uint32(0xFFFFF000)).view(np.float32)
    lo = (a - hi).astype(np.float32)
    return hi, lo


def _make_in_maps(x, w1):
    """x: (B, 4096, 3) float32, w1: (64, 6) -> per-core input dicts.

    lt rows (K=16):
      k0-11 : 2*y_piece[n] (pieces h,h,l,l x c=0..2)
      k12-13: xx_n pieces
      k14-15: -1
    rt = [rtS | rtD] (16, 2048+2048): the S columns make the matmul emit
    S[n,j] = dist[n,j] + dist[n,j+2048]; the D columns emit
    D[n,j] = dist[n,j] - dist[n,j+2048]. All rhs entries are re-split to
    <=12 significant bits so every fp32r product stays exact.

    xquad rows (1024, 24): [Y_j Y_j+1024 Y_j+2048 Y_j+3072 | x_... same].
    w3 (3, 192): [w1x.T | w1d.T | (w1x-w1d).T] for the folded conv.
    """
    H = N // 2
    Q = N // 4
    w1 = np.ascontiguousarray(w1, np.float32)
    w6 = np.ascontiguousarray(w1.T)
    iota16 = np.ascontiguousarray(
        np.tile(np.arange(16, dtype=np.float32), (P, 1)))
    in_maps = []
    for b in range(B):
        xb = np.ascontiguousarray(x[b], dtype=np.float32)       # (4096, 3)
        y = xb.reshape(3, N)                                     # flat view
        xx = np.sum(y * y, axis=0, dtype=np.float32)             # (4096,)
        yh, yl = _split_fp32r(y)
        xh, xl = _split_fp32r(xx)
        lt = np.empty((16, N), np.float32)
        for i, la in enumerate([yh, yh, yl, yl]):
            lt[3 * i:3 * i + 3] = 2.0 * la
        lt[12], lt[13] = xh, xl
        lt[14], lt[15] = -1.0, -1.0

        s = (y[:, :H] + y[:, H:]).astype(np.float32)
        dd = (y[:, :H] - y[:, H:]).astype(np.float32)
        sh, sl = _split_fp32r(s)
        dh, dl = _split_fp32r(dd)
        xxs = (xx[:H] + xx[H:]).astype(np.float32)
        xxd = (xx[:H] - xx[H:]).astype(np.float32)
        xxs_h, xxs_l = _split_fp32r(xxs)
        xxd_h, xxd_l = _split_fp32r(xxd)
        rt = np.empty((16, N), np.float32)
        for i, ra in enumerate([sh, sl, sh, sl]):
            rt[3 * i:3 * i + 3, :H] = ra
        for i, ra in enumerate([dh, dl, dh, dl]):
            rt[3 * i:3 * i + 3, H:] = ra
        rt[12:14, :H] = -2.0
        rt[14, :H], rt[15, :H] = xxs_h, xxs_l
        rt[12:14, H:] = 0.0
        rt[14, H:], rt[15, H:] = xxd_h, xxd_l

        Y = np.ascontiguousarray(y.T, np.float32)              # (4096, 3) col coords
        xquad = np.ascontiguousarray(np.concatenate(
            [Y[0:Q], Y[Q:2 * Q], Y[2 * Q:3 * Q], Y[3 * Q:4 * Q],
             xb[0:Q], xb[Q:2 * Q], xb[2 * Q:3 * Q], xb[3 * Q:4 * Q]],
            axis=1), np.float32)
        in_maps.append({"lt": lt, "rt": rt, "xrows": xb, "ycols": Y,
                        "xquad": xquad, "iota16": iota16, "w6": w6})
    return in_maps


def run_device(x, w1, trace=False):
    """Run the per-sample stage-1 kernel on 8 cores.

    Returns (seg_max (B, 64, 3) float32, exec_time_ns or None).
    """
    from concourse.bass_utils import run_bass_kernel_spmd

    nc = _get_nc()
    in_maps = _make_in_maps(x, w1)
    core_ids = list(range(B))
    r = run_bass_kernel_spmd(nc, in_maps, core_ids, trace=trace)
    seg = np.stack([np.asarray(r.results[i]["out"]) for i in range(B)])
    return seg, r.exec_time_ns


# ---------------- host-side downstream (mirrors reference.py in fp32) -------

def _topk_idx(dist, k):
    # lax.top_k: descending values, ties -> lowest index first
    return np.argsort(-dist, axis=-1, kind="stable")[..., :k]


def _get_graph_feature_np(x, k):
    Bb, Nn = x.shape[0], x.shape[1]
    x = x.reshape(Bb, -1, Nn)
    C = x.shape[1]
    inner = (-2.0 * np.einsum("bcn,bcm->bnm", x, x)).astype(np.float32)
    xx = np.sum(x * x, axis=1, keepdims=True, dtype=np.float32)
    dist = -xx - inner - np.swapaxes(xx, 1, 2)
    idx = _topk_idx(dist, k)
    flat = x.reshape(Bb * Nn, C)
    idx_full = (idx + (np.arange(Bb) * Nn)[:, None, None]).reshape(-1)
    feature = flat[idx_full].reshape(Bb, Nn, k, C)
    xc = x.reshape(Bb, Nn, 1, C)
    feature = np.concatenate(
        [feature - xc, np.broadcast_to(xc, (Bb, Nn, k, C))], axis=3
    )
    return feature.reshape(Bb, 2 * C, k, Nn)


def _bn(h, g, b, m, v, axis):
    shape = [1] * h.ndim
    shape[axis] = -1
    inv = (g.reshape(shape) / np.sqrt(v.reshape(shape) + EPS)).astype(np.float32)
    return ((h - m.reshape(shape)) * inv + b.reshape(shape)).astype(np.float32)


def _leaky(x):
    return np.where(x >= 0, x, np.float32(0.2) * x).astype(np.float32)


def _softmax(z):
    z = z - np.max(z, axis=1, keepdims=True)
    e = np.exp(z)
    return (e / np.sum(e, axis=1, keepdims=True)).astype(np.float32)


def kernel(x, w1, wA, bA, wB, bB, wC, bC,
           bn1_g, bn1_b, bn1_m, bn1_v,
           bnA_g, bnA_b, bnA_m, bnA_v,
           bnB_g, bnB_b, bnB_m, bnB_v):
    x = np.asarray(x, np.float32)
    w1 = np.asarray(w1, np.float32)

    seg, _ = run_device(x, w1)                               # (B, 64, 3) raw maxima
    x1 = _leaky(_bn(seg, np.asarray(bn1_g, np.float32), np.asarray(bn1_b, np.float32),
                    np.asarray(bn1_m, np.float32), np.asarray(bn1_v, np.float32), 1))
    x2 = np.max(_get_graph_feature_np(x1, K), axis=-1)
    x3 = np.max(_get_graph_feature_np(x2, K), axis=-1)
    x4 = np.max(_get_graph_feature_np(x3, K), axis=-1)
    xc = np.concatenate([x1, x2, x3, x4], axis=1)            # (B, 82, 3)
    p = np.max(xc, axis=-1)                                  # (B, 82)
    h = _leaky(_bn(p @ np.asarray(wA, np.float32).T + np.asarray(bA, np.float32),
                   np.asarray(bnA_g, np.float32), np.asarray(bnA_b, np.float32),
                   np.asarray(bnA_m, np.float32), np.asarray(bnA_v, np.float32), 1))
    h = _leaky(_bn(h @ np.asarray(wB, np.float32).T + np.asarray(bB, np.float32),
                   np.asarray(bnB_g, np.float32), np.asarray(bnB_b, np.float32),
                   np.asarray(bnB_m, np.float32), np.asarray(bnB_v, np.float32), 1))
    return _softmax(h @ np.asarray(wC, np.float32).T + np.asarray(bC, np.float32))


# revision 20
# speedup vs baseline: 1.3771x; 1.1302x over previous
"""DGCNN forward kernel for Trainium2 (8 NeuronCores, data-parallel over batch).

Contract: kernel(**inputs) takes the FULL unsharded inputs (keyed as in
setup_inputs()) and returns the FULL (8, 3) float32 output.

Strategy
--------
B = 8 samples -> 1 sample per NeuronCore (pure data parallel; the tiny weights
are replicated). Per sample, the dominant work is stage 1 of the DGCNN:

  y     = x.reshape(3, 4096)            (flat view, matches the torch .view)
  dist  = (2*y^T y - xx_n) - xx_m       (4096 x 4096)
  idx   = top-3 columns per row         (includes self)
  x1    = leaky(bn1(max_n w1 @ [x[idx]-x[n]; x[n]]))   -> (64, 3)

Everything through the segment-max runs on device; bn1+leaky (monotone, so it
commutes with the max bit-exactly) and the later KNN stages on 64/6-point
clouds + the tiny MLP (~2 KFLOP total, <0.01% of the FLOPs) run on host in
float32, mirroring the reference ops exactly.

Distance matmul: operands are split exactly into hi+lo FP32R pieces (<=12
significant bits each, so every product is exact) and folded into one K=16
FP32R matmul per 512-column chunk, accumulated in PSUM in the reference's
rounding order.

Selection uses a QUAD-REDUCTION: columns {j, j+1024, j+2048, j+3072} form a
group. The PE emits S = distA+distB (cols 0:2048) and D = distA-distB (cols
2048:4096) for the (j, j+2048) pairs; ACT evicts 0.5*S (Copy) and 0.5*|D|
(Abs); one gpsimd CCE DMA (accum add) forms the pair key
m[j] = max(distA, distB); one DVE tensor_max folds pairs (j, j+1024) into
quad keys q[j] = max of the 4 group members. The DVE top-8/index scans then
run on 1024 keys instead of 4096 (the scans are the DVE bottleneck; they are
elem-count-bound at ~1/cycle regardless of dtype).

Exactness: the top-3 quads by q provably contain {self, nn1, nn2} (a quad
outranks nn2's quad only if it holds self or nn1). All 4 members of the top-3
quads are gathered as 96B rows [Ycol x4 | xrow x4] (SWDGE); the refine
recomputes exact squared distances from the FLAT-VIEW COLUMN coords Y (the
reference's distance space) while features use the flat-view ROW coords x
(the reference's gather space). The self candidate has d = -0.0 exactly, so
it is always refine-rank 0; ranks 1,2 are the k=1,2 neighbors, selected by a
one-hot sum over 12 slots. Numpy-validated: 0/32768 neighbor mismatches vs
the fp32 reference on the test data.

The reference conv contracts w1 against a FLAT .view() of the n-major
edge-feature stream (g.reshape(6, 12288) crosses point boundaries), so the
per-tile g rows ([d_kk(3) | x_n(3)] x 3, with d exactly fp32-subtracted)
round-trip through a DRAM scratch buffer whose flat re-view feeds plain
fp32 K=6 matmuls (exact; the PE has slack). Chunk maxima respect the k'
quarters and fold into the (64, 3) segment maxima at the end.

Per 128-row tile, software-pipelined (stage2 lags stage1 by LAG=3 tiles):
  PE    : 8 K=16 FP32R dist matmuls -> PSUM (S|D), fp32 conv matmuls
  ACT   : PSUM evict (Copy 0.5*S, Abs 0.5*D), refine square, g x-blocks
  DMA   : gpsimd CCE add m += tt; SWDGE candidate gathers; g stream store+load
  DVE   : quad tensor_max + InstMax/InstMaxIndex on q (1024) + refine scans
          + conv reduce_max
  GPSIMD: gathers + refine smalls (sub, one-hot select)
"""

import numpy as np

N = 4096
P = 128
NT = N // P           # 32 row tiles
B = 8
EPS = 1e-5
K = 3

_compiled = None


def _build(reps=1, ablate=None):
    # ablate: None=full, "dist"=PE+ACT only, "m"=+gpsimd pair-max,
    #         "q"=+DVE quad-max, "max"=+InstMax, "idx"=+InstMaxIndex,
    #         "cand"=+gathers, "dm"=+refine dist, "dsf"=+refine scans,
    #         "xs"=+one-hot select (no conv/transpose)
    import contextlib
    HCDBG = globals().get('HCDBG_COL', 128)

    import concourse.bass as bass
    import concourse.mybir as mybir
    from concourse import bacc
    from concourse.tile import TileContext

    f32 = mybir.dt.float32
    f32r = mybir.dt.float32r
    u32 = mybir.dt.uint32
    Copy = mybir.ActivationFunctionType.Copy
    H = N // 2
    Q = N // 4

    nc = bacc.Bacc(
        "TRN2", target_bir_lowering=False, debug=False, num_devices=B
    )
    lt = nc.declare_dram_parameter("lt", [16, N], f32r, isOutput=False)
    rt = nc.declare_dram_parameter("rt", [16, N], f32r, isOutput=False)
    xr = nc.declare_dram_parameter("xrows", [N, 3], f32, isOutput=False)
    xq = nc.declare_dram_parameter("xquad", [Q, 24], f32, isOutput=False)
    yc = nc.declare_dram_parameter("ycols", [N, 3], f32, isOutput=False)
    io16 = nc.declare_dram_parameter("iota16", [P, 16], f32, isOutput=False)
    w6p = nc.declare_dram_parameter("w6", [6, 64], f32r, isOutput=False)
    out_p = nc.declare_dram_parameter("out", [64, 3], f32, isOutput=True)

    g_sep = nc.dram_tensor("g_scratch", [N, 18], f32r)

    with TileContext(nc) as tc:
        with (
            tc.tile_pool(name="const", bufs=1) as cpool,
            tc.tile_pool(name="dist", bufs=3) as dpool,
            tc.tile_pool(name="mkey", bufs=4) as mpool,
            tc.tile_pool(name="work", bufs=8) as wpool,
        ):
            lt_sb = cpool.tile([16, N], f32r)
            nc.sync.dma_start(out=lt_sb[:, 0:256], in_=lt[:, 0:256])
            nc.sync.dma_start(out=lt_sb[:, 256:N], in_=lt[:, 256:N])
            rt_sb = cpool.tile([16, N], f32r)
            for cchunk in range(4):
                nc.sync.dma_start(
                    out=rt_sb[:, cchunk * 1024:(cchunk + 1) * 1024],
                    in_=rt[:, cchunk * 1024:(cchunk + 1) * 1024],
                )
            w6_sb = cpool.tile([6, 64], f32r)
            nc.sync.dma_start(out=w6_sb[:], in_=w6p[:])
            xall = cpool.tile([P, NT, 3], f32)
            nc.sync.dma_start(
                out=xall[:],
                in_=xr[:].rearrange("(t p) c -> p t c", p=P),
            )
            yall = cpool.tile([P, NT, 3], f32)
            nc.sync.dma_start(
                out=yall[:],
                in_=yc[:].rearrange("(t p) c -> p t c", p=P),
            )
            iota16 = cpool.tile([P, 16], f32)
            nc.sync.dma_start(out=iota16[:], in_=io16[:])

            loop_cm = tc.For_i(0, reps, 1) if reps > 1 else contextlib.nullcontext()
            with loop_cm:
              with tc.tile_pool(name="psum_d", bufs=3, space="PSUM") as ppool, \
                   tc.tile_pool(name="psum_c", bufs=1, space="PSUM") as cppool:
                partial2a = wpool.tile([64, 24], f32, tag="partial2a")
                # pre-initialize ring-buffer slots the steady-state loop
                # never writes: dm[:, 12:16] = -inf pads, g[:, 0:3] = 0
                # (the kk=0 edge feature is identically zero).
                if ablate in (None, "xs", "dsf", "dm"):
                    for _ in range(8):
                        dm0 = wpool.tile([P, 16], f32, tag="dm")
                        nc.vector.memset(dm0[:, 12:16], -3.0e38)
                if ablate in (None, "xs"):
                    for _ in range(8):
                        g0 = wpool.tile([P, 18], f32r, tag="g")
                        nc.vector.memset(g0[:, 0:3].bitcast(u32), 0)

                m32s, cands, difs, sqs, scans_out, sels, prodss = {}, {}, {}, {}, {}, {}, {}

                def stage1a(t):
                    """dist matmuls -> ACT evict (S, |D|) -> CCE pair key."""
                    m32 = mpool.tile([P, H], f32, tag="m32")
                    tt = dpool.tile([P, H], f32, tag="dist")
                    for h in range(4):
                        ps = ppool.tile([P, 1024], f32, tag="ps")
                        for j in range(2):
                            col0 = h * 1024 + j * 512
                            nc.tensor.matmul(
                                out=ps[:, j * 512:(j + 1) * 512],
                                lhsT=lt_sb[:, t * P:(t + 1) * P],
                                rhs=rt_sb[:, col0:col0 + 512],
                                start=True,
                                stop=True,
                            )
                        dst = (m32[:, h * 1024:(h + 1) * 1024] if h < 2
                               else tt[:, (h - 2) * 1024:(h - 1) * 1024])
                        nc.scalar.activation(
                            out=dst,
                            in_=ps[:],
                            func=(Copy if h < 2
                                  else mybir.ActivationFunctionType.Abs),
                            scale=0.5,
                        )
                    if ablate == "dist":
                        if t == NT - 1:
                            res0 = wpool.tile([P, 1], f32, tag="res0")
                            nc.vector.reduce_max(
                                out=res0[:], in_=tt[:],
                                axis=mybir.AxisListType.X)
                            nc.sync.dma_start(out=out_p[0:64, 0:1],
                                              in_=res0[0:64, :])
                        return
                    nc.gpsimd.dma_start(out=m32[:], in_=tt[:],
                                        accum_op=mybir.AluOpType.add)
                    if ablate == "m":
                        if t == NT - 1:
                            res0 = wpool.tile([P, 1], f32, tag="res0")
                            nc.vector.reduce_max(
                                out=res0[:], in_=m32[:],
                                axis=mybir.AxisListType.X)
                            nc.sync.dma_start(out=out_p[0:64, 0:1],
                                              in_=res0[0:64, :])
                        return
                    m32s[t] = m32

                def stage1b(t):
                    """quad fold -> top-8 scan -> candidate gathers."""
                    m32 = m32s.pop(t)
                    qk = mpool.tile([P, Q], f32, tag="qk")
                    nc.vector.tensor_max(out=qk[:], in0=m32[:, 0:Q],
                                         in1=m32[:, Q:H])
                    if ablate == "q":
                        if t == NT - 1:
                            res0 = wpool.tile([P, 1], f32, tag="res0")
                            nc.vector.reduce_max(
                                out=res0[:], in_=qk[:],
                                axis=mybir.AxisListType.X)
                            nc.sync.dma_start(out=out_p[0:64, 0:1],
                                              in_=res0[0:64, :])
                        return
                    maxv = wpool.tile([P, 8], f32, tag="maxv")
                    nc.vector.max(out=maxv[:], in_=qk[:])
                    idxs = wpool.tile([P, 8], u32, tag="idxs")
                    nc.vector.max_index(out=idxs[:], in_max=maxv[:],
                                        in_values=qk[:])
                    if ablate == "idx":
                        if t == NT - 1:
                            idf = wpool.tile([P, 3], f32, tag="idf")
                            nc.vector.tensor_copy(out=idf[:], in_=idxs[:, 0:3])
                            nc.sync.dma_start(out=out_p[0:64, 0:3],
                                              in_=idf[0:64, :])
                        return
                    candp = wpool.tile([P, 3, 24], f32, tag="cand")
                    sp0 = 128 * (t % 8)
                    nc.sync.dma_start(out=candp[:, 0, :],
                                      in_=xq[sp0:sp0 + 128, :])
                    for s in range(1, 3):
                        nc.gpsimd.indirect_dma_start(
                            out=candp[:, s, :],
                            out_offset=None,
                            in_=xq[:],
                            in_offset=bass.IndirectOffsetOnAxis(
                                ap=idxs[:, s:s + 1], axis=0
                            ),
                        )
                    if ablate == "cand":
                        if t == NT - 1:
                            nc.sync.dma_start(out=out_p[0:64, 0:3],
                                              in_=candp[0:64, 0, 12:15])
                        return
                    cands[t] = candp

                def stage2a(t):
                    """refine distances: dif (gpsimd)."""
                    candp = cands[t]
                    cand_d = candp[:, :, 0:12].rearrange(
                        "p a (m c) -> p a m c", m=4)          # (P, 3, 4, 3)
                    dif = wpool.tile([P, 3, 4, 3], f32, tag="dif")
                    nc.gpsimd.tensor_sub(
                        out=dif[:], in0=cand_d,
                        in1=yall[:, t:t + 1, :].rearrange(
                            "p (a o) c -> p a o c", o=1
                        ).to_broadcast([P, 3, 4, 3]),
                    )
                    difs[t] = dif

                def stage2b(t):
                    """refine distances: square (ACT)."""
                    dif = difs.pop(t)
                    sq = wpool.tile([P, 3, 4, 3], f32, tag="sq")
                    nc.scalar.square(out=sq[:], in_=dif[:])
                    sqs[t] = sq

                def stage2c(t):
                    """refine reduce + top-3-of-12 scan (DVE)."""
                    sq = sqs.pop(t)
                    dm = wpool.tile([P, 16], f32, tag="dm")
                    nc.vector.tensor_reduce(
                        out=dm[:, 0:12].rearrange("p (a m) -> p a m", m=4),
                        in_=sq[:], axis=mybir.AxisListType.X,
                        op=mybir.AluOpType.add, negate=True,
                    )
                    if ablate == "dm":
                        if t == NT - 1:
                            nc.sync.dma_start(out=out_p[0:64, 0:3],
                                              in_=dm[0:64, 0:3])
                        return
                    dv = wpool.tile([P, 8], f32, tag="dv")
                    nc.vector.max(out=dv[:], in_=dm[:])
                    dslots = wpool.tile([P, 8], u32, tag="dslots")
                    nc.vector.max_index(out=dslots[:], in_max=dv[:],
                                        in_values=dm[:])
                    scans_out[t] = dslots

                def stage2d(t):
                    """one-hot select of the k=1,2 neighbors (gpsimd)."""
                    dslots = scans_out.pop(t)
                    candp = cands.pop(t)
                    dsf = wpool.tile([P, 2], f32, tag="dsf")
                    nc.gpsimd.tensor_copy(out=dsf[:], in_=dslots[:, 1:3])
                    if ablate == "dsf":
                        if t == NT - 1:
                            dsl = wpool.tile([P, 3], f32, tag="dsl")
                            nc.vector.tensor_copy(out=dsl[:],
                                                  in_=dslots[:, 0:3])
                            nc.sync.dma_start(out=out_p[0:64, 0:3],
                                              in_=dsl[0:64, :])
                        return
                    oh = wpool.tile([P, 2, 16], f32, tag="oh")
                    for k in range(2):
                        nc.gpsimd.tensor_scalar(
                            out=oh[:, k, :], in0=iota16[:],
                            scalar1=dsf[:, k:k + 1], scalar2=None,
                            op0=mybir.AluOpType.is_equal,
                        )
                    cand_x = candp[:, :, 12:24].rearrange(
                        "p a (m c) -> p a m c", m=4)          # (P, 3, 4, 3)
                    # edge vectors d = x_cand - x_n computed exactly in fp32
                    # BEFORE the one-hot sum (keeps fp32r rounding relative
                    # to the small d, not the O(1) coords).
                    cand_xd = wpool.tile([P, 3, 4, 3], f32, tag="cand_xd")
                    nc.gpsimd.tensor_sub(
                        out=cand_xd[:], in0=cand_x,
                        in1=xall[:, t:t + 1, :].rearrange(
                            "p (a o) c -> p a o c", o=1
                        ).to_broadcast([P, 3, 4, 3]),
                    )
                    prods = wpool.tile([P, 2, 3, 4, 3], f32, tag="prods")
                    for k in range(2):
                        nc.gpsimd.tensor_mul(
                            out=prods[:, k],
                            in0=cand_xd[:],
                            in1=oh[:, k, 0:12].rearrange(
                                "p (a m o) -> p a m o", m=4, o=1
                            ).to_broadcast([P, 3, 4, 3]),
                        )
                    prodss[t] = prods

                def stage2e(t):
                    """g assembly -> stream store -> gated conv chunks."""
                    prods = prodss.pop(t)
                    g = wpool.tile([P, 18], f32r, tag="g")
                    g3 = g[:].rearrange("p (a b) -> p a b", a=3)
                    nc.scalar.activation(
                        out=g3[:, :, 3:6],
                        in_=xall[:, t:t + 1, :].to_broadcast([P, 3, 3]),
                        func=Copy,
                    )
                    with nc.allow_low_precision(
                            reason="f32r SBUF stores full fp32 bits"):
                        nc.vector.tensor_reduce(
                            out=g3[:, 1:3, 0:3],
                            in_=prods[:].rearrange("p k a m c -> p k c (a m)"),
                            axis=mybir.AxisListType.X,
                            op=mybir.AluOpType.add,
                        )
                    if ablate == "xs":
                        if t == NT - 1:
                            nc.sync.dma_start(out=out_p[0:64, 0:3],
                                              in_=g[0:64, 6:9].bitcast(f32))
                        return
                    nc.sync.dma_start(
                        out=g_sep[t * P:(t + 1) * P, :], in_=g[:])
                    for m in range(12):
                        c_hi = 2 * m + 1
                        n_max = (5 * 12288 + 512 * (c_hi + 1) - 1) // 18
                        gate = n_max // P
                        if gate != t:
                            continue
                        psc = cppool.tile([64, 2, 512], f32, tag="pst")
                        for half in range(2):
                            c = 2 * m + half
                            g24c = wpool.tile([6, 512], f32r, tag="g24c")
                            hlv = g_sep[:].flatten().rearrange(
                                "(x b) -> x b", x=6)[:, c * 512:(c + 1) * 512]
                            nc.sync.dma_start(out=g24c[:], in_=hlv)
                            nc.tensor.matmul(
                                out=psc[:, half, :],
                                lhsT=w6_sb[:],
                                rhs=g24c[:],
                                start=True,
                                stop=True,
                            )
                        nc.vector.reduce_max(
                            out=partial2a[:, 2 * m:2 * m + 2],
                            in_=psc[:],
                            axis=mybir.AxisListType.X,
                        )

                # depth-6 software pipeline: each cross-engine dependency is
                # at least one iteration old, so no engine queue head ever
                # waits on work emitted in the same iteration. Deepest stage
                # first so ready work sits at each queue head.
                stages = [(6, stage2e), (5, stage2d), (4, stage2c),
                          (3, stage2b), (2, stage2a), (1, stage1b),
                          (0, stage1a)]
                cut = {"dist": 0, "m": 0, "q": 1, "max": 1, "idx": 1,
                       "cand": 1, "dm": 4, "dsf": 5, "xs": 6}.get(ablate, 6)
                for it in range(NT + 6):
                    for lag, fn in stages:
                        if lag > cut:
                            continue
                        tt_ = it - lag
                        if 0 <= tt_ < NT:
                            fn(tt_)

              if ablate is None:
                res = wpool.tile([64, 3], f32, tag="res")
                nc.vector.reduce_max(
                    out=res[:],
                    in_=partial2a[:].rearrange("p (k g) -> p k g", k=3),
                    axis=mybir.AxisListType.X,
                )
                nc.sync.dma_start(out=out_p[:], in_=res[:])
    nc.compile()
    return nc


def _get_nc():
    global _compiled
    if _compiled is None:
        _compiled = _build()
    return _compiled


def _split_fp32r(a):
    """Exact split a = hi + lo with both pieces having <= 12 significant bits."""
    a = np.ascontiguousarray(a, np.float32)
    hi = (a.view(np.uint32) & np.uint32(0xFFFFF000)).view(np.float32)
    lo = (a - hi).astype(np.float32)
    return hi, lo


def _make_in_maps(x, w1):
    """x: (B, 4096, 3) float32, w1: (64, 6) -> per-core input dicts.

    lt rows (K=16):
      k0-11 : 2*y_piece[n] (pieces h,h,l,l x c=0..2)
      k12-13: xx_n pieces
      k14-15: -1
    rt = [rtS | rtD] (16, 2048+2048): the S columns make the matmul emit
    S[n,j] = dist[n,j] + dist[n,j+2048]; the D columns emit
    D[n,j] = dist[n,j] - dist[n,j+2048]. All rhs entries are re-split to
    <=12 significant bits so every fp32r product stays exact.

    xquad rows (1024, 24): [Y_j Y_j+1024 Y_j+2048 Y_j+3072 | x_... same].
    w3 (3, 192): [w1x.T | w1d.T | (w1x-w1d).T] for the folded conv.
    """
    H = N // 2
    Q = N // 4
    w1 = np.ascontiguousarray(w1, np.float32)
    w6 = np.ascontiguousarray(w1.T)
    iota16 = np.ascontiguousarray(
        np.tile(np.arange(16, dtype=np.float32), (P, 1)))
    in_maps = []
    for b in range(B):
        xb = np.ascontiguousarray(x[b], dtype=np.float32)       # (4096, 3)
        y = xb.reshape(3, N)                                     # flat view
        xx = np.sum(y * y, axis=0, dtype=np.float32)             # (4096,)
        yh, yl = _split_fp32r(y)
        xh, xl = _split_fp32r(xx)
        lt = np.empty((16, N), np.float32)
        for i, la in enumerate([yh, yh, yl, yl]):
            lt[3 * i:3 * i + 3] = 2.0 * la
        lt[12], lt[13] = xh, xl
        lt[14], lt[15] = -1.0, -1.0

        s = (y[:, :H] + y[:, H:]).astype(np.float32)
        dd = (y[:, :H] - y[:, H:]).astype(np.float32)
        sh, sl = _split_fp32r(s)
        dh, dl = _split_fp32r(dd)
        xxs = (xx[:H] + xx[H:]).astype(np.float32)
        xxd = (xx[:H] - xx[H:]).astype(np.float32)
        xxs_h, xxs_l = _split_fp32r(xxs)
        xxd_h, xxd_l = _split_fp32r(xxd)
        rt = np.empty((16, N), np.float32)
        for i, ra in enumerate([sh, sl, sh, sl]):
            rt[3 * i:3 * i + 3, :H] = ra
        for i, ra in enumerate([dh, dl, dh, dl]):
            rt[3 * i:3 * i + 3, H:] = ra
        rt[12:14, :H] = -2.0
        rt[14, :H], rt[15, :H] = xxs_h, xxs_l
        rt[12:14, H:] = 0.0
        rt[14, H:], rt[15, H:] = xxd_h, xxd_l

        Y = np.ascontiguousarray(y.T, np.float32)              # (4096, 3) col coords
        xquad = np.ascontiguousarray(np.concatenate(
            [Y[0:Q], Y[Q:2 * Q], Y[2 * Q:3 * Q], Y[3 * Q:4 * Q],
             xb[0:Q], xb[Q:2 * Q], xb[2 * Q:3 * Q], xb[3 * Q:4 * Q]],
            axis=1), np.float32)
        in_maps.append({"lt": lt, "rt": rt, "xrows": xb, "ycols": Y,
                        "xquad": xquad, "iota16": iota16, "w6": w6})
    return in_maps


def run_device(x, w1, trace=False):
    """Run the per-sample stage-1 kernel on 8 cores.

    Returns (seg_max (B, 64, 3) float32, exec_time_ns or None).
    """
    from concourse.bass_utils import run_bass_kernel_spmd

    nc = _get_nc()
    in_maps = _make_in_maps(x, w1)
    core_ids = list(range(B))
    r = run_bass_kernel_spmd(nc, in_maps, core_ids, trace=trace)
    seg = np.stack([np.asarray(r.results[i]["out"]) for i in range(B)])
    return seg, r.exec_time_ns


# ---------------- host-side downstream (mirrors reference.py in fp32) -------

def _topk_idx(dist, k):
    # lax.top_k: descending values, ties -> lowest index first
    return np.argsort(-dist, axis=-1, kind="stable")[..., :k]


def _get_graph_feature_np(x, k):
    Bb, Nn = x.shape[0], x.shape[1]
    x = x.reshape(Bb, -1, Nn)
    C = x.shape[1]
    inner = (-2.0 * np.einsum("bcn,bcm->bnm", x, x)).astype(np.float32)
    xx = np.sum(x * x, axis=1, keepdims=True, dtype=np.float32)
    dist = -xx - inner - np.swapaxes(xx, 1, 2)
    idx = _topk_idx(dist, k)
    flat = x.reshape(Bb * Nn, C)
    idx_full = (idx + (np.arange(Bb) * Nn)[:, None, None]).reshape(-1)
    feature = flat[idx_full].reshape(Bb, Nn, k, C)
    xc = x.reshape(Bb, Nn, 1, C)
    feature = np.concatenate(
        [feature - xc, np.broadcast_to(xc, (Bb, Nn, k, C))], axis=3
    )
    return feature.reshape(Bb, 2 * C, k, Nn)


def _bn(h, g, b, m, v, axis):
    shape = [1] * h.ndim
    shape[axis] = -1
    inv = (g.reshape(shape) / np.sqrt(v.reshape(shape) + EPS)).astype(np.float32)
    return ((h - m.reshape(shape)) * inv + b.reshape(shape)).astype(np.float32)


def _leaky(x):
    return np.where(x >= 0, x, np.float32(0.2) * x).astype(np.float32)


def _softmax(z):
    z = z - np.max(z, axis=1, keepdims=True)
    e = np.exp(z)
    return (e / np.sum(e, axis=1, keepdims=True)).astype(np.float32)


def kernel(x, w1, wA, bA, wB, bB, wC, bC,
           bn1_g, bn1_b, bn1_m, bn1_v,
           bnA_g, bnA_b, bnA_m, bnA_v,
           bnB_g, bnB_b, bnB_m, bnB_v):
    x = np.asarray(x, np.float32)
    w1 = np.asarray(w1, np.float32)

    seg, _ = run_device(x, w1)                               # (B, 64, 3) raw maxima
    x1 = _leaky(_bn(seg, np.asarray(bn1_g, np.float32), np.asarray(bn1_b, np.float32),
                    np.asarray(bn1_m, np.float32), np.asarray(bn1_v, np.float32), 1))
    x2 = np.max(_get_graph_feature_np(x1, K), axis=-1)
    x3 = np.max(_get_graph_feature_np(x2, K), axis=-1)
    x4 = np.max(_get_graph_feature_np(x3, K), axis=-1)
    xc = np.concatenate([x1, x2, x3, x4], axis=1)            # (B, 82, 3)
    p = np.max(xc, axis=-1)                                  # (B, 82)
    h = _leaky(_bn(p @ np.asarray(wA, np.float32).T + np.asarray(bA, np.float32),
                   np.asarray(bnA_g, np.float32), np.asarray(bnA_b, np.float32),
                   np.asarray(bnA_m, np.float32), np.asarray(bnA_v, np.float32), 1))
    h = _leaky(_bn(h @ np.asarray(wB, np.float32).T + np.asarray(bB, np.float32),
                   np.asarray(bnB_g, np.float32), np.asarray(bnB_b, np.float32),
                   np.asarray(bnB_m, np.float32), np.asarray(bnB_v, np.float32), 1))
    return _softmax(h @ np.asarray(wC, np.float32).T + np.asarray(bC, np.float32))


# revision 21
# speedup vs baseline: 1.4462x; 1.0502x over previous
"""DGCNN forward kernel for Trainium2 (8 NeuronCores, data-parallel over batch).

Contract: kernel(**inputs) takes the FULL unsharded inputs (keyed as in
setup_inputs()) and returns the FULL (8, 3) float32 output.

Strategy
--------
B = 8 samples -> 1 sample per NeuronCore (pure data parallel; the tiny weights
are replicated). Per sample, the dominant work is stage 1 of the DGCNN:

  y     = x.reshape(3, 4096)            (flat view, matches the torch .view)
  dist  = (2*y^T y - xx_n) - xx_m       (4096 x 4096)
  idx   = top-3 columns per row         (includes self)
  x1    = leaky(bn1(max_n w1 @ [x[idx]-x[n]; x[n]]))   -> (64, 3)

Everything through the segment-max runs on device; bn1+leaky (monotone, so it
commutes with the max bit-exactly) and the later KNN stages on 64/6-point
clouds + the tiny MLP (~2 KFLOP total, <0.01% of the FLOPs) run on host in
float32, mirroring the reference ops exactly.

Distance matmul: operands are split exactly into hi+lo FP32R pieces (<=12
significant bits each, so every product is exact) and folded into one K=16
FP32R matmul per 512-column chunk, accumulated in PSUM in the reference's
rounding order.

Selection uses a QUAD-REDUCTION: columns {j, j+1024, j+2048, j+3072} form a
group. The PE emits S = distA+distB (cols 0:2048) and D = distA-distB (cols
2048:4096) for the (j, j+2048) pairs; ACT evicts 0.5*S (Copy) and 0.5*|D|
(Abs); one gpsimd CCE DMA (accum add) forms the pair key
m[j] = max(distA, distB); one DVE tensor_max folds pairs (j, j+1024) into
quad keys q[j] = max of the 4 group members. The DVE top-8/index scans then
run on 1024 keys instead of 4096 (the scans are the DVE bottleneck; they are
elem-count-bound at ~1/cycle regardless of dtype).

Exactness: the top-3 quads by q provably contain {self, nn1, nn2} (a quad
outranks nn2's quad only if it holds self or nn1). All 4 members of the top-3
quads are gathered as 96B rows [Ycol x4 | xrow x4] (SWDGE); the refine
recomputes exact squared distances from the FLAT-VIEW COLUMN coords Y (the
reference's distance space) while features use the flat-view ROW coords x
(the reference's gather space). The self candidate has d = -0.0 exactly, so
it is always refine-rank 0; ranks 1,2 are the k=1,2 neighbors, selected by a
one-hot sum over 12 slots. Numpy-validated: 0/32768 neighbor mismatches vs
the fp32 reference on the test data.

The reference conv contracts w1 against a FLAT .view() of the n-major
edge-feature stream (g.reshape(6, 12288) crosses point boundaries), so the
per-tile g rows ([d_kk(3) | x_n(3)] x 3, with d exactly fp32-subtracted)
round-trip through a DRAM scratch buffer whose flat re-view feeds plain
fp32 K=6 matmuls (exact; the PE has slack). Chunk maxima respect the k'
quarters and fold into the (64, 3) segment maxima at the end.

Per 128-row tile, software-pipelined (stage2 lags stage1 by LAG=3 tiles):
  PE    : 8 K=16 FP32R dist matmuls -> PSUM (S|D), fp32 conv matmuls
  ACT   : PSUM evict (Copy 0.5*S, Abs 0.5*D), refine square, g x-blocks
  DMA   : gpsimd CCE add m += tt; SWDGE candidate gathers; g stream store+load
  DVE   : quad tensor_max + InstMax/InstMaxIndex on q (1024) + refine scans
          + conv reduce_max
  GPSIMD: gathers + refine smalls (sub, one-hot select)
"""

import numpy as np

N = 4096
P = 128
NT = N // P           # 32 row tiles
B = 8
EPS = 1e-5
K = 3

_compiled = None


def _build(reps=1, ablate=None):
    # ablate: None=full, "dist"=PE+ACT only, "m"=+gpsimd pair-max,
    #         "q"=+DVE quad-max, "max"=+InstMax, "idx"=+InstMaxIndex,
    #         "cand"=+gathers, "dm"=+refine dist, "dsf"=+refine scans,
    #         "xs"=+one-hot select (no conv/transpose)
    import contextlib
    HCDBG = globals().get('HCDBG_COL', 128)

    import concourse.bass as bass
    import concourse.mybir as mybir
    from concourse import bacc
    from concourse.tile import TileContext

    f32 = mybir.dt.float32
    f32r = mybir.dt.float32r
    u32 = mybir.dt.uint32
    Copy = mybir.ActivationFunctionType.Copy
    H = N // 2
    Q = N // 4

    nc = bacc.Bacc(
        "TRN2", target_bir_lowering=False, debug=False, num_devices=B
    )
    lt = nc.declare_dram_parameter("lt", [16, N], f32r, isOutput=False)
    rt = nc.declare_dram_parameter("rt", [16, N], f32r, isOutput=False)
    xr = nc.declare_dram_parameter("xrows", [N, 3], f32, isOutput=False)
    xq = nc.declare_dram_parameter("xquad", [Q, 24], f32, isOutput=False)
    yc = nc.declare_dram_parameter("ycols", [N, 3], f32, isOutput=False)
    io16 = nc.declare_dram_parameter("iota16", [P, 16], f32, isOutput=False)
    w6p = nc.declare_dram_parameter("w6", [6, 64], f32r, isOutput=False)
    out_p = nc.declare_dram_parameter("out", [64, 3], f32, isOutput=True)

    g_sep = nc.dram_tensor("g_scratch", [N, 18], f32r)

    with TileContext(nc) as tc:
        with (
            tc.tile_pool(name="const", bufs=1) as cpool,
            tc.tile_pool(name="dist", bufs=3) as dpool,
            tc.tile_pool(name="mkey", bufs=4) as mpool,
            tc.tile_pool(name="work", bufs=8) as wpool,
        ):
            lt_sb = cpool.tile([16, N], f32r)
            nc.sync.dma_start(out=lt_sb[:, 0:256], in_=lt[:, 0:256])
            nc.sync.dma_start(out=lt_sb[:, 256:N], in_=lt[:, 256:N])
            rt_sb = cpool.tile([16, N], f32r)
            for cchunk in range(4):
                nc.sync.dma_start(
                    out=rt_sb[:, cchunk * 1024:(cchunk + 1) * 1024],
                    in_=rt[:, cchunk * 1024:(cchunk + 1) * 1024],
                )
            w6_sb = cpool.tile([6, 64], f32r)
            nc.sync.dma_start(out=w6_sb[:], in_=w6p[:])
            xall = cpool.tile([P, NT, 3], f32)
            nc.sync.dma_start(
                out=xall[:],
                in_=xr[:].rearrange("(t p) c -> p t c", p=P),
            )
            yall = cpool.tile([P, NT, 3], f32)
            nc.sync.dma_start(
                out=yall[:],
                in_=yc[:].rearrange("(t p) c -> p t c", p=P),
            )
            iota16 = cpool.tile([P, 16], f32)
            nc.sync.dma_start(out=iota16[:], in_=io16[:])

            loop_cm = tc.For_i(0, reps, 1) if reps > 1 else contextlib.nullcontext()
            with loop_cm:
              with tc.tile_pool(name="psum_d", bufs=3, space="PSUM") as ppool, \
                   tc.tile_pool(name="psum_c", bufs=1, space="PSUM") as cppool:
                partial2a = wpool.tile([64, 24], f32, tag="partial2a")
                # pre-initialize ring-buffer slots the steady-state loop
                # never writes: dm[:, 12:16] = -inf pads, g[:, 0:3] = 0
                # (the kk=0 edge feature is identically zero).
                if ablate in (None, "xs", "dsf", "dm"):
                    for _ in range(8):
                        dm0 = wpool.tile([P, 16], f32, tag="dm")
                        nc.vector.memset(dm0[:, 12:16], -3.0e38)
                if ablate in (None, "xs"):
                    for _ in range(8):
                        g0 = wpool.tile([P, 18], f32r, tag="g")
                        nc.vector.memset(g0[:, 0:3].bitcast(u32), 0)

                m32s, cands, difs, sqs, scans_out, sels, prodss = {}, {}, {}, {}, {}, {}, {}

                def stage1a(t):
                    """dist matmuls -> ACT evict (S, |D|) -> CCE pair key."""
                    m32 = mpool.tile([P, H], f32, tag="m32")
                    tt = dpool.tile([P, H], f32, tag="dist")
                    for h in range(4):
                        ps = ppool.tile([P, 1024], f32, tag="ps")
                        for j in range(2):
                            col0 = h * 1024 + j * 512
                            nc.tensor.matmul(
                                out=ps[:, j * 512:(j + 1) * 512],
                                lhsT=lt_sb[:, t * P:(t + 1) * P],
                                rhs=rt_sb[:, col0:col0 + 512],
                                start=True,
                                stop=True,
                            )
                        dst = (m32[:, h * 1024:(h + 1) * 1024] if h < 2
                               else tt[:, (h - 2) * 1024:(h - 1) * 1024])
                        nc.scalar.activation(
                            out=dst,
                            in_=ps[:],
                            func=(Copy if h < 2
                                  else mybir.ActivationFunctionType.Abs),
                            scale=0.5,
                        )
                    if ablate == "dist":
                        if t == NT - 1:
                            res0 = wpool.tile([P, 1], f32, tag="res0")
                            nc.vector.reduce_max(
                                out=res0[:], in_=tt[:],
                                axis=mybir.AxisListType.X)
                            nc.sync.dma_start(out=out_p[0:64, 0:1],
                                              in_=res0[0:64, :])
                        return
                    nc.gpsimd.dma_start(out=m32[:], in_=tt[:],
                                        accum_op=mybir.AluOpType.add,
                                        single_packet=True)
                    if ablate == "m":
                        if t == NT - 1:
                            res0 = wpool.tile([P, 1], f32, tag="res0")
                            nc.vector.reduce_max(
                                out=res0[:], in_=m32[:],
                                axis=mybir.AxisListType.X)
                            nc.sync.dma_start(out=out_p[0:64, 0:1],
                                              in_=res0[0:64, :])
                        return
                    m32s[t] = m32

                def stage1b(t):
                    """quad fold -> top-8 scan -> candidate gathers."""
                    m32 = m32s.pop(t)
                    qk = mpool.tile([P, Q], f32, tag="qk")
                    nc.vector.tensor_max(out=qk[:], in0=m32[:, 0:Q],
                                         in1=m32[:, Q:H])
                    if ablate == "q":
                        if t == NT - 1:
                            res0 = wpool.tile([P, 1], f32, tag="res0")
                            nc.vector.reduce_max(
                                out=res0[:], in_=qk[:],
                                axis=mybir.AxisListType.X)
                            nc.sync.dma_start(out=out_p[0:64, 0:1],
                                              in_=res0[0:64, :])
                        return
                    maxv = wpool.tile([P, 8], f32, tag="maxv")
                    nc.vector.max(out=maxv[:], in_=qk[:])
                    idxs = wpool.tile([P, 8], u32, tag="idxs")
                    nc.vector.max_index(out=idxs[:], in_max=maxv[:],
                                        in_values=qk[:])
                    if ablate == "idx":
                        if t == NT - 1:
                            idf = wpool.tile([P, 3], f32, tag="idf")
                            nc.vector.tensor_copy(out=idf[:], in_=idxs[:, 0:3])
                            nc.sync.dma_start(out=out_p[0:64, 0:3],
                                              in_=idf[0:64, :])
                        return
                    candp = wpool.tile([P, 3, 24], f32, tag="cand")
                    sp0 = 128 * (t % 8)
                    nc.sync.dma_start(out=candp[:, 0, :],
                                      in_=xq[sp0:sp0 + 128, :])
                    for s in range(1, 3):
                        nc.gpsimd.indirect_dma_start(
                            out=candp[:, s, :],
                            out_offset=None,
                            in_=xq[:],
                            in_offset=bass.IndirectOffsetOnAxis(
                                ap=idxs[:, s:s + 1], axis=0
                            ),
                        )
                    if ablate == "cand":
                        if t == NT - 1:
                            nc.sync.dma_start(out=out_p[0:64, 0:3],
                                              in_=candp[0:64, 0, 12:15])
                        return
                    cands[t] = candp

                def stage2a(t):
                    """refine distances: dif (gpsimd)."""
                    candp = cands[t]
                    cand_d = candp[:, :, 0:12].rearrange(
                        "p a (m c) -> p a m c", m=4)          # (P, 3, 4, 3)
                    dif = wpool.tile([P, 3, 4, 3], f32, tag="dif")
                    nc.gpsimd.tensor_sub(
                        out=dif[:], in0=cand_d,
                        in1=yall[:, t:t + 1, :].rearrange(
                            "p (a o) c -> p a o c", o=1
                        ).to_broadcast([P, 3, 4, 3]),
                    )
                    difs[t] = dif

                def stage2b(t):
                    """refine distances: square (ACT)."""
                    dif = difs.pop(t)
                    sq = wpool.tile([P, 3, 4, 3], f32, tag="sq")
                    nc.scalar.square(out=sq[:], in_=dif[:])
                    sqs[t] = sq

                def stage2c(t):
                    """refine reduce + top-3-of-12 scan (DVE)."""
                    sq = sqs.pop(t)
                    dm = wpool.tile([P, 16], f32, tag="dm")
                    nc.vector.tensor_reduce(
                        out=dm[:, 0:12].rearrange("p (a m) -> p a m", m=4),
                        in_=sq[:], axis=mybir.AxisListType.X,
                        op=mybir.AluOpType.add, negate=True,
                    )
                    if ablate == "dm":
                        if t == NT - 1:
                            nc.sync.dma_start(out=out_p[0:64, 0:3],
                                              in_=dm[0:64, 0:3])
                        return
                    dv = wpool.tile([P, 8], f32, tag="dv")
                    nc.vector.max(out=dv[:], in_=dm[:])
                    dslots = wpool.tile([P, 8], u32, tag="dslots")
                    nc.vector.max_index(out=dslots[:], in_max=dv[:],
                                        in_values=dm[:])
                    scans_out[t] = dslots

                def stage2d(t):
                    """one-hot select of the k=1,2 neighbors (gpsimd)."""
                    dslots = scans_out.pop(t)
                    candp = cands.pop(t)
                    dsf = wpool.tile([P, 2], f32, tag="dsf")
                    nc.gpsimd.tensor_copy(out=dsf[:], in_=dslots[:, 1:3])
                    if ablate == "dsf":
                        if t == NT - 1:
                            dsl = wpool.tile([P, 3], f32, tag="dsl")
                            nc.vector.tensor_copy(out=dsl[:],
                                                  in_=dslots[:, 0:3])
                            nc.sync.dma_start(out=out_p[0:64, 0:3],
                                              in_=dsl[0:64, :])
                        return
                    oh = wpool.tile([P, 2, 16], f32, tag="oh")
                    for k in range(2):
                        nc.gpsimd.tensor_scalar(
                            out=oh[:, k, :], in0=iota16[:],
                            scalar1=dsf[:, k:k + 1], scalar2=None,
                            op0=mybir.AluOpType.is_equal,
                        )
                    cand_x = candp[:, :, 12:24].rearrange(
                        "p a (m c) -> p a m c", m=4)          # (P, 3, 4, 3)
                    # edge vectors d = x_cand - x_n computed exactly in fp32
                    # BEFORE the one-hot sum (keeps fp32r rounding relative
                    # to the small d, not the O(1) coords).
                    cand_xd = wpool.tile([P, 3, 4, 3], f32, tag="cand_xd")
                    nc.gpsimd.tensor_sub(
                        out=cand_xd[:], in0=cand_x,
                        in1=xall[:, t:t + 1, :].rearrange(
                            "p (a o) c -> p a o c", o=1
                        ).to_broadcast([P, 3, 4, 3]),
                    )
                    prods = wpool.tile([P, 2, 3, 4, 3], f32, tag="prods")
                    for k in range(2):
                        nc.gpsimd.tensor_mul(
                            out=prods[:, k],
                            in0=cand_xd[:],
                            in1=oh[:, k, 0:12].rearrange(
                                "p (a m o) -> p a m o", m=4, o=1
                            ).to_broadcast([P, 3, 4, 3]),
                        )
                    prodss[t] = prods

                def stage2e(t):
                    """g assembly -> stream store -> gated conv chunks."""
                    prods = prodss.pop(t)
                    g = wpool.tile([P, 18], f32r, tag="g")
                    g3 = g[:].rearrange("p (a b) -> p a b", a=3)
                    nc.scalar.activation(
                        out=g3[:, :, 3:6],
                        in_=xall[:, t:t + 1, :].to_broadcast([P, 3, 3]),
                        func=Copy,
                    )
                    with nc.allow_low_precision(
                            reason="f32r SBUF stores full fp32 bits"):
                        nc.vector.tensor_reduce(
                            out=g3[:, 1:3, 0:3],
                            in_=prods[:].rearrange("p k a m c -> p k c (a m)"),
                            axis=mybir.AxisListType.X,
                            op=mybir.AluOpType.add,
                        )
                    if ablate == "xs":
                        if t == NT - 1:
                            nc.sync.dma_start(out=out_p[0:64, 0:3],
                                              in_=g[0:64, 6:9].bitcast(f32))
                        return
                    nc.sync.dma_start(
                        out=g_sep[t * P:(t + 1) * P, :], in_=g[:])
                    for m in range(12):
                        c_hi = 2 * m + 1
                        n_max = (5 * 12288 + 512 * (c_hi + 1) - 1) // 18
                        gate = n_max // P
                        if gate != t:
                            continue
                        psc = cppool.tile([64, 2, 512], f32, tag="pst")
                        for half in range(2):
                            c = 2 * m + half
                            g24c = wpool.tile([6, 512], f32r, tag="g24c")
                            hlv = g_sep[:].flatten().rearrange(
                                "(x b) -> x b", x=6)[:, c * 512:(c + 1) * 512]
                            nc.sync.dma_start(out=g24c[:], in_=hlv)
                            nc.tensor.matmul(
                                out=psc[:, half, :],
                                lhsT=w6_sb[:],
                                rhs=g24c[:],
                                start=True,
                                stop=True,
                            )
                        nc.vector.reduce_max(
                            out=partial2a[:, 2 * m:2 * m + 2],
                            in_=psc[:],
                            axis=mybir.AxisListType.X,
                        )

                # depth-6 software pipeline: each cross-engine dependency is
                # at least one iteration old, so no engine queue head ever
                # waits on work emitted in the same iteration. Deepest stage
                # first so ready work sits at each queue head.
                stages = [(6, stage2e), (5, stage2d), (4, stage2c),
                          (3, stage2b), (2, stage2a), (1, stage1b),
                          (0, stage1a)]
                cut = {"dist": 0, "m": 0, "q": 1, "max": 1, "idx": 1,
                       "cand": 1, "dm": 4, "dsf": 5, "xs": 6}.get(ablate, 6)
                for it in range(NT + 6):
                    for lag, fn in stages:
                        if lag > cut:
                            continue
                        tt_ = it - lag
                        if 0 <= tt_ < NT:
                            fn(tt_)

              if ablate is None:
                res = wpool.tile([64, 3], f32, tag="res")
                nc.vector.reduce_max(
                    out=res[:],
                    in_=partial2a[:].rearrange("p (k g) -> p k g", k=3),
                    axis=mybir.AxisListType.X,
                )
                nc.sync.dma_start(out=out_p[:], in_=res[:])
    nc.compile()
    return nc


def _get_nc():
    global _compiled
    if _compiled is None:
        _compiled = _build()
    return _compiled


def _split_fp32r(a):
    """Exact split a = hi + lo with both pieces having <= 12 significant bits."""
    a = np.ascontiguousarray(a, np.float32)
    hi = (a.view(np.uint32) & np.uint32(0xFFFFF000)).view(np.float32)
    lo = (a - hi).astype(np.float32)
    return hi, lo


def _make_in_maps(x, w1):
    """x: (B, 4096, 3) float32, w1: (64, 6) -> per-core input dicts.

    lt rows (K=16):
      k0-11 : 2*y_piece[n] (pieces h,h,l,l x c=0..2)
      k12-13: xx_n pieces
      k14-15: -1
    rt = [rtS | rtD] (16, 2048+2048): the S columns make the matmul emit
    S[n,j] = dist[n,j] + dist[n,j+2048]; the D columns emit
    D[n,j] = dist[n,j] - dist[n,j+2048]. All rhs entries are re-split to
    <=12 significant bits so every fp32r product stays exact.

    xquad rows (1024, 24): [Y_j Y_j+1024 Y_j+2048 Y_j+3072 | x_... same].
    w3 (3, 192): [w1x.T | w1d.T | (w1x-w1d).T] for the folded conv.
    """
    H = N // 2
    Q = N // 4
    w1 = np.ascontiguousarray(w1, np.float32)
    w6 = np.ascontiguousarray(w1.T)
    iota16 = np.ascontiguousarray(
        np.tile(np.arange(16, dtype=np.float32), (P, 1)))
    in_maps = []
    for b in range(B):
        xb = np.ascontiguousarray(x[b], dtype=np.float32)       # (4096, 3)
        y = xb.reshape(3, N)                                     # flat view
        xx = np.sum(y * y, axis=0, dtype=np.float32)             # (4096,)
        yh, yl = _split_fp32r(y)
        xh, xl = _split_fp32r(xx)
        lt = np.empty((16, N), np.float32)
        for i, la in enumerate([yh, yh, yl, yl]):
            lt[3 * i:3 * i + 3] = 2.0 * la
        lt[12], lt[13] = xh, xl
        lt[14], lt[15] = -1.0, -1.0

        s = (y[:, :H] + y[:, H:]).astype(np.float32)
        dd = (y[:, :H] - y[:, H:]).astype(np.float32)
        sh, sl = _split_fp32r(s)
        dh, dl = _split_fp32r(dd)
        xxs = (xx[:H] + xx[H:]).astype(np.float32)
        xxd = (xx[:H] - xx[H:]).astype(np.float32)
        xxs_h, xxs_l = _split_fp32r(xxs)
        xxd_h, xxd_l = _split_fp32r(xxd)
        rt = np.empty((16, N), np.float32)
        for i, ra in enumerate([sh, sl, sh, sl]):
            rt[3 * i:3 * i + 3, :H] = ra
        for i, ra in enumerate([dh, dl, dh, dl]):
            rt[3 * i:3 * i + 3, H:] = ra
        rt[12:14, :H] = -2.0
        rt[14, :H], rt[15, :H] = xxs_h, xxs_l
        rt[12:14, H:] = 0.0
        rt[14, H:], rt[15, H:] = xxd_h, xxd_l

        Y = np.ascontiguousarray(y.T, np.float32)              # (4096, 3) col coords
        xquad = np.ascontiguousarray(np.concatenate(
            [Y[0:Q], Y[Q:2 * Q], Y[2 * Q:3 * Q], Y[3 * Q:4 * Q],
             xb[0:Q], xb[Q:2 * Q], xb[2 * Q:3 * Q], xb[3 * Q:4 * Q]],
            axis=1), np.float32)
        in_maps.append({"lt": lt, "rt": rt, "xrows": xb, "ycols": Y,
                        "xquad": xquad, "iota16": iota16, "w6": w6})
    return in_maps


def run_device(x, w1, trace=False):
    """Run the per-sample stage-1 kernel on 8 cores.

    Returns (seg_max (B, 64, 3) float32, exec_time_ns or None).
    """
    from concourse.bass_utils import run_bass_kernel_spmd

    nc = _get_nc()
    in_maps = _make_in_maps(x, w1)
    core_ids = list(range(B))
    r = run_bass_kernel_spmd(nc, in_maps, core_ids, trace=trace)
    seg = np.stack([np.asarray(r.results[i]["out"]) for i in range(B)])
    return seg, r.exec_time_ns


# ---------------- host-side downstream (mirrors reference.py in fp32) -------

def _topk_idx(dist, k):
    # lax.top_k: descending values, ties -> lowest index first
    return np.argsort(-dist, axis=-1, kind="stable")[..., :k]


def _get_graph_feature_np(x, k):
    Bb, Nn = x.shape[0], x.shape[1]
    x = x.reshape(Bb, -1, Nn)
    C = x.shape[1]
    inner = (-2.0 * np.einsum("bcn,bcm->bnm", x, x)).astype(np.float32)
    xx = np.sum(x * x, axis=1, keepdims=True, dtype=np.float32)
    dist = -xx - inner - np.swapaxes(xx, 1, 2)
    idx = _topk_idx(dist, k)
    flat = x.reshape(Bb * Nn, C)
    idx_full = (idx + (np.arange(Bb) * Nn)[:, None, None]).reshape(-1)
    feature = flat[idx_full].reshape(Bb, Nn, k, C)
    xc = x.reshape(Bb, Nn, 1, C)
    feature = np.concatenate(
        [feature - xc, np.broadcast_to(xc, (Bb, Nn, k, C))], axis=3
    )
    return feature.reshape(Bb, 2 * C, k, Nn)


def _bn(h, g, b, m, v, axis):
    shape = [1] * h.ndim
    shape[axis] = -1
    inv = (g.reshape(shape) / np.sqrt(v.reshape(shape) + EPS)).astype(np.float32)
    return ((h - m.reshape(shape)) * inv + b.reshape(shape)).astype(np.float32)


def _leaky(x):
    return np.where(x >= 0, x, np.float32(0.2) * x).astype(np.float32)


def _softmax(z):
    z = z - np.max(z, axis=1, keepdims=True)
    e = np.exp(z)
    return (e / np.sum(e, axis=1, keepdims=True)).astype(np.float32)


def kernel(x, w1, wA, bA, wB, bB, wC, bC,
           bn1_g, bn1_b, bn1_m, bn1_v,
           bnA_g, bnA_b, bnA_m, bnA_v,
           bnB_g, bnB_b, bnB_m, bnB_v):
    x = np.asarray(x, np.float32)
    w1 = np.asarray(w1, np.float32)

    seg, _ = run_device(x, w1)                               # (B, 64, 3) raw maxima
    x1 = _leaky(_bn(seg, np.asarray(bn1_g, np.float32), np.asarray(bn1_b, np.float32),
                    np.asarray(bn1_m, np.float32), np.asarray(bn1_v, np.float32), 1))
    x2 = np.max(_get_graph_feature_np(x1, K), axis=-1)
    x3 = np.max(_get_graph_feature_np(x2, K), axis=-1)
    x4 = np.max(_get_graph_feature_np(x3, K), axis=-1)
    xc = np.concatenate([x1, x2, x3, x4], axis=1)            # (B, 82, 3)
    p = np.max(xc, axis=-1)                                  # (B, 82)
    h = _leaky(_bn(p @ np.asarray(wA, np.float32).T + np.asarray(bA, np.float32),
                   np.asarray(bnA_g, np.float32), np.asarray(bnA_b, np.float32),
                   np.asarray(bnA_m, np.float32), np.asarray(bnA_v, np.float32), 1))
    h = _leaky(_bn(h @ np.asarray(wB, np.float32).T + np.asarray(bB, np.float32),
                   np.asarray(bnB_g, np.float32), np.asarray(bnB_b, np.float32),
                   np.asarray(bnB_m, np.float32), np.asarray(bnB_v, np.float32), 1))
    return _softmax(h @ np.asarray(wC, np.float32).T + np.asarray(bC, np.float32))
